# revision 23
# baseline (speedup 1.0000x reference)
"""Gemma2 sliding-window attention (B=1, S=4096, HID=3584, 16 Q heads / 8 KV heads,
HD=256, window 2047, tanh softcap 50) on 8 Trainium2 NeuronCores.

Sharding: tensor-parallel over heads with NO on-device collectives. Core c owns
Q heads (2c, 2c+1) and KV head c, and computes a full-shape PARTIAL of the
output projection restricted to its own 512 attention features:
    partial_c = attn[:, 512c:512c+512] @ w_o[:, 512c:512c+512].T   [S, HID]
The host sums the 8 bf16 partials in float64 (unshard of the sum-sharded
output). This removes the AllGather + serial o-proj tail.

Per-core fused pipeline over 512-token tiles tt=0..7:
  A(tt): QKV projection (transposed for Q/K, straight for V) + NeoX RoPE.
         The normalization chain of block tt-1 (denominator matmuls -> fast
         approximate reciprocals -> partition broadcasts -> ao multiplies) is
         injected between A's projection groups, where the tensor queue is
         deep and the DVE dacc chain has long drained - it never stalls
         anything.
  B(tt): sliding-window attention for query block tt with BOTH heads merged
         per key-subtile: one combined [128, 2, 512] PSUM score tile (the
         head axis is the bank boundary), ONE batched tanh and ONE batched
         exp over both heads (amortizes the 352-cycle ACT instruction
         overhead - ACT was the B-phase bottleneck engine), shared K/V
         stationary operands, and per-element-has_written column-restricted
         boundary tiles (saves ~15% of score/PV/tanh/exp work). The
         denominator accumulates on DVE in bf16 (2x mode). o-proj chunks of
         block tt-1 interleave into the loop; their PSUM->SBUF copies ride
         the DVE (ACT stays reserved for the softmax chain).

Pipelining details:
  - Block tt's first two score batches are pre-emitted into A(tt)'s tail so
    their tanh/exp chain hides under the V-projection matmuls (no block-entry
    ACT bubble).
  - V-projection PSUMs ride the psO ring (free during A after the previous
    block's normalization reads), keeping the psS ring exclusively for the
    projection/score/o-proj rotation.
  - RoPE multiplies are ordered so each projection PSUM buffer is released
    after 2 DVE ops instead of 5.
  - o-proj chunk pacing finishes two iterations before each block ends so
    the next A phase never waits on a chunk-copy drain.
Startup: tile-0 DMAs are issued in just-in-time consumption order (hid/wqk
ko-chunked and interleaved on the sync ring; cos/sin/masks/hid-half1 on the
scalar HWDGE ring) so the first matmul starts ~4us in. The final o-proj
block alternates its copies across ACT and DVE (both idle by then) to
shorten the tail.

Measured on hardware: 848 us (baseline of this session: 994 us), rel err
7.3e-3 vs the f64 reference.
"""

import sys

if "/opt/trn_rl_repo" not in sys.path:
    sys.path.insert(0, "/opt/trn_rl_repo")

import numpy as np
import ml_dtypes

import concourse.bass as bass
import concourse.tile as tile
from concourse import bacc, mybir
from concourse.bass_utils import run_bass_kernel_spmd

# Problem constants (hardcoded per harness contract)
S = 4096
HID = 3584
NH, NKV, HD = 16, 8, 256
Q_SIZE = NH * HD          # 4096
SCALE = 256.0 ** -0.5     # 1/16
SOFTCAP = 50.0
WINDOW = 2048 - 1         # 2047
THETA = 10000.0

N_CORES = 8
KO = HID // 128           # 28 contraction subtiles for projections
TT = S // 512             # 8 token tiles of 512
HC = HID // 512           # 7 output-column chunks of 512
F32 = mybir.dt.float32
BF16 = mybir.dt.bfloat16

# Boundary-tile diagonal offsets (q0 - 128*kt). Interior iff 128 <= off <= 1536.
MASK_OFFS = [-384, -256, -128, 0, 1664, 1792, 1920, 2048]

_NC_CACHE = {}


def build_nc():
    nc = bacc.Bacc()

    # All inputs are host-side pre-arranged to [128 partitions, ...contiguous]
    # so every DMA is 128 large contiguous descriptors.
    hidT_r = nc.declare_dram_parameter("hidTp", [128, 2 * TT, KO, 256], BF16,
                                       isOutput=False)
    wqkT_r = nc.declare_dram_parameter("wqkTp", [128, 3, KO, 256], BF16,
                                       isOutput=False)
    wvT_r = nc.declare_dram_parameter("wvTp", [128, KO, HD], BF16,
                                      isOutput=False)
    woT_r = nc.declare_dram_parameter("woTp", [128, 4, HID], BF16,
                                      isOutput=False)
    cosT = nc.declare_dram_parameter("cosT", [128, S], F32, isOutput=False)
    sinT = nc.declare_dram_parameter("sinT", [128, S], F32, isOutput=False)
    masks_r = nc.declare_dram_parameter("masksp", [128, 8, 512], BF16,
                                        isOutput=False)
    out = nc.declare_dram_parameter("out", [S, HID], BF16, isOutput=True)

    with tile.TileContext(nc) as tc:
        with (
            tc.tile_pool(name="persist", bufs=1) as persist,
            tc.tile_pool(name="hidp", bufs=2) as hid_pool,
            tc.tile_pool(name="cs", bufs=2) as cs_pool,
            tc.tile_pool(name="qp", bufs=2) as q_pool,
            tc.tile_pool(name="rp", bufs=4) as rp_pool,
            tc.tile_pool(name="probs", bufs=4) as probs_pool,
            tc.tile_pool(name="aop", bufs=8) as ao_pool,
            tc.tile_pool(name="otp", bufs=5) as out_pool,
            tc.tile_pool(name="small", bufs=2) as small_pool,
            tc.tile_pool(name="dap", bufs=2) as dacc_pool,
            tc.tile_pool(name="psS", bufs=2, space="PSUM") as psS,
            tc.tile_pool(name="psO", bufs=4, space="PSUM") as psO,
        ):
            # Persistent SBUF: weights, per-tile K/V, masks, ones.
            wqk_sb = persist.tile([128, 3, KO, 256], BF16, tag="wqk")
            wv_sb = persist.tile([128, KO, HD], BF16, tag="wv")
            mask_sb = persist.tile([128, 8, 512], BF16, tag="mask")
            wo_sb = persist.tile([128, 4, HID], BF16, tag="wo")
            ones_bf = persist.tile([128, 1], BF16, tag="ones")
            nc.vector.memset(ones_bf, 1.0)
            # HAM warm-up: dependency-free dummy matmuls keep the PE busy
            # through the startup DMA waits so the real projections run at
            # the unthrottled 2.4 GHz clock from the start
            warm_sb = persist.tile([128, 512], BF16, tag="warm")
            nc.vector.memset(warm_sb, 0.0)
            warm_ps = psO.tile([128, 512], F32, tag="po", name="warm")
            for _ in range(16):
                nc.tensor.matmul(warm_ps, warm_sb[:, 0:128], warm_sb,
                                 start=True, stop=True,
                                 skip_group_check=True)

            k_sb = [persist.tile([128, 2, 512], BF16, tag=f"k{t}", name=f"k{t}")
                    for t in range(TT)]
            v_sb = [persist.tile([128, 4, HD], BF16, tag=f"v{t}", name=f"v{t}")
                    for t in range(TT)]

            ao_store = {}
            norm_src = {}
            norm_cell = {}

            def emit_norm_stage1(qb):
                """Denominator matmul + fast reciprocal + broadcast for both
                heads of block qb. Called mid-A(qb+1): the tensor queue is
                deep with projection matmuls, so the pden matmuls never
                head-of-line block, and the dacc chains drained long ago."""
                for h in (0, 1):
                    po_lo, po_hi, dacc_h = norm_src[(qb, h)]
                    pden = psS.tile([1, 512], F32, tag="s", name="pden")
                    nc.tensor.matmul(pden, ones_bf, dacc_h,
                                     start=True, stop=True,
                                     skip_group_check=True)
                    recip = small_pool.tile([1, 512], F32, tag="recip",
                                            name="recip")
                    nc.vector.reciprocal_approx_fast(recip, pden)
                    rb = small_pool.tile([128, 512], F32, tag="rb", name="rb")
                    nc.gpsimd.partition_broadcast(rb, recip)
                    norm_cell[h] = rb

            def emit_norm_stage2(qb):
                for h in (0, 1):
                    po_lo, po_hi, _ = norm_src.pop((qb, h))
                    rb = norm_cell.pop(h)
                    ao0 = ao_pool.tile([128, 512], BF16, tag="ao", name="ao0")
                    ao1 = ao_pool.tile([128, 512], BF16, tag="ao", name="ao1")
                    nc.vector.tensor_mul(ao0, po_lo, rb)
                    nc.vector.tensor_mul(ao1, po_hi, rb)
                    ao_store[(qb, h)] = (ao0, ao1)

            def prefetch(tt):
                """Issue hid/cos/sin DMAs for tile tt. Tile 0 gets a
                just-in-time order: hid/wqk ko-chunked and interleaved so the
                first matmul group is gated on ~1.4 MB, not 6 MB."""
                hids = []
                if tt == 0:
                    hid_h0 = hid_pool.tile([128, KO, 256], BF16, tag="hid",
                                           name="hid_h")
                    # fine-grained ko-interleave: each 7-ko chunk of hid and
                    # wqk pair 0 lands just before the projection consumes it
                    for c0 in range(0, KO, 7):
                        c1 = min(c0 + 7, KO)
                        nc.sync.dma_start(hid_h0[:, c0:c1, :],
                                          hidT_r[:, 0, c0:c1, :])
                        nc.sync.dma_start(wqk_sb[:, 0, c0:c1, :],
                                          wqkT_r[:, 0, c0:c1, :])
                    nc.sync.dma_start(wqk_sb[:, 1, 0:14, :],
                                      wqkT_r[:, 1, 0:14, :])
                    nc.sync.dma_start(wqk_sb[:, 1, 14:KO, :],
                                      wqkT_r[:, 1, 14:KO, :])
                    hids.append(hid_h0)
                    # small/late tile-0 transfers ride the second HWDGE
                    # ring (scalar): the sync-ring weight stream lands sooner
                    cos_t = cs_pool.tile([128, 512], F32, tag="cos",
                                         name="cos_t")
                    nc.scalar.dma_start(cos_t, cosT[:, bass.ts(tt, 512)])
                    sin_t = cs_pool.tile([128, 512], F32, tag="sin",
                                         name="sin_t")
                    nc.scalar.dma_start(sin_t, sinT[:, bass.ts(tt, 512)])
                    # second hid half + remaining weights are emitted JIT
                    # inside emit_A(0)
                    hid_h1 = hid_pool.tile([128, KO, 256], BF16, tag="hid",
                                           name="hid_h")
                    hids.append(hid_h1)
                    return hids, cos_t, sin_t, True
                for half in range(2):
                    hid_h = hid_pool.tile([128, KO, 256], BF16, tag="hid",
                                          name="hid_h")
                    nc.sync.dma_start(hid_h, hidT_r[:, tt * 2 + half, :, :])
                    hids.append(hid_h)
                cos_t = cs_pool.tile([128, 512], F32, tag="cos", name="cos_t")
                nc.sync.dma_start(cos_t, cosT[:, bass.ts(tt, 512)])
                sin_t = cs_pool.tile([128, 512], F32, tag="sin", name="sin_t")
                nc.sync.dma_start(sin_t, sinT[:, bass.ts(tt, 512)])
                return hids, cos_t, sin_t, False

            def make_B_state(qb, q_t):
                """Closures + state for block qb's merged-head attention.
                scores() is usable as soon as q RoPE is done, so the first
                two batches can be pre-emitted into A's tail (their tanh/exp
                hide under the V-projection matmuls, removing the block-entry
                ACT bubble)."""
                q0 = qb * 512
                kts = list(range(max(0, 4 * qb - 16), 4 * qb + 4))
                n = len(kts)
                dacc = dacc_pool.tile([128, 2, 512], BF16, tag="da",
                                      name="dacc")
                nc.vector.memset(dacc, 0.0)
                probs = {}
                st = dict(qb=qb, kts=kts, n=n, dacc=dacc, probs=probs,
                          emitted=0, pos_t=None)

                def scores(i):
                    kt = kts[i]
                    off = q0 - 128 * kt
                    lo, hi = max(0, -off), min(512, 2176 - off)
                    ttk, ksub = kt // 4, kt % 4
                    ksl = bass.ts(ksub, 128)
                    ps = psS.tile([128, 2, 512], F32, tag="s", name="ps_s")
                    nc.tensor.matmul(ps[:, 0, lo:hi], k_sb[ttk][:, 0, ksl],
                                     q_t[:, 0, lo:hi], start=True, stop=False)
                    nc.tensor.matmul(ps[:, 1, lo:hi], k_sb[ttk][:, 0, ksl],
                                     q_t[:, 2, lo:hi], start=True, stop=False)
                    nc.tensor.matmul(ps[:, 0, lo:hi], k_sb[ttk][:, 1, ksl],
                                     q_t[:, 1, lo:hi], start=False, stop=True)
                    nc.tensor.matmul(ps[:, 1, lo:hi], k_sb[ttk][:, 1, ksl],
                                     q_t[:, 3, lo:hi], start=False, stop=True)
                    pt = probs_pool.tile([128, 2, 512], BF16, tag="pt",
                                         name="pt")
                    nc.scalar.activation(
                        ps[:, :, lo:hi], ps[:, :, lo:hi],
                        mybir.ActivationFunctionType.Tanh,
                        scale=SCALE / SOFTCAP,
                    )
                    nc.scalar.activation(
                        pt[:, :, lo:hi], ps[:, :, lo:hi],
                        mybir.ActivationFunctionType.Exp,
                        scale=SOFTCAP,
                    )
                    if not (128 <= off <= 1536):
                        mi = MASK_OFFS.index(off)
                        for hh in (0, 1):
                            nc.vector.tensor_mul(pt[:, hh, lo:hi],
                                                 pt[:, hh, lo:hi],
                                                 mask_sb[:, mi, lo:hi])
                    if lo == 0 and hi == 512:
                        nc.vector.tensor_add(dacc, dacc, pt)
                    else:
                        for hh in (0, 1):
                            nc.vector.tensor_add(dacc[:, hh, lo:hi],
                                                 dacc[:, hh, lo:hi],
                                                 pt[:, hh, lo:hi])
                    probs[i] = pt

                st["scores"] = scores
                return st

            def emit_A(tt, pre):
                """QKV projection + RoPE for token tile tt. Returns the B
                state for block tt. Block tt-1's normalization stages are
                injected between projection groups of the first half; block
                tt's first two score batches are pre-emitted before the
                second half's V-projection groups."""
                hids, cos_t, sin_t, jit0 = pre
                q_t = q_pool.tile([128, 4, 512], BF16, tag="q", name="q_t")
                bst = None
                for half in range(2):
                    csl = bass.ts(half, 256)
                    hid_h = hids[half]
                    for pair in range(3):
                        if jit0 and half == 0 and pair == 1:
                            # fill the pair0->pair1 weight-DMA wait
                            for _ in range(20):
                                nc.tensor.matmul(warm_ps, warm_sb[:, 0:128],
                                                 warm_sb, start=True,
                                                 stop=True,
                                                 skip_group_check=True)
                        if jit0 and half == 0 and pair == 2:
                            nc.sync.dma_start(wqk_sb[:, 2, 0:14, :],
                                              wqkT_r[:, 2, 0:14, :])
                            nc.sync.dma_start(wqk_sb[:, 2, 14:KO, :],
                                              wqkT_r[:, 2, 14:KO, :])
                            # fill the pair1->pair2 weight-DMA wait
                            for _ in range(28):
                                nc.tensor.matmul(warm_ps, warm_sb[:, 0:128],
                                                 warm_sb, start=True,
                                                 stop=True,
                                                 skip_group_check=True)
                        if half == 0 and pair == 2 and tt > 0:
                            emit_norm_stage1(tt - 1)
                        ps_a = psS.tile([128, 256], F32, tag="s", name="ps_a")
                        for ko in range(KO):
                            nc.tensor.matmul(
                                ps_a,
                                wqk_sb[:, pair, ko, 0:128],
                                hid_h[:, ko, :],
                                start=(ko == 0), stop=(ko == KO - 1),
                            )
                        ps_b = psS.tile([128, 256], F32, tag="s", name="ps_b")
                        for ko in range(KO):
                            nc.tensor.matmul(
                                ps_b,
                                wqk_sb[:, pair, ko, 128:256],
                                hid_h[:, ko, :],
                                start=(ko == 0), stop=(ko == KO - 1),
                            )
                        if pair < 2:
                            d1 = q_t[:, 2 * pair, csl]
                            d2 = q_t[:, 2 * pair + 1, csl]
                        else:
                            d1 = k_sb[tt][:, 0, csl]
                            d2 = k_sb[tt][:, 1, csl]
                        # ps_a is read by the first two DVE ops and ps_b by
                        # the next two, so each PSUM ring buffer frees after
                        # 2 ops instead of 5 - the next projection group's
                        # matmuls unblock ~1.5us earlier
                        t1 = rp_pool.tile([128, 256], F32, tag="rp", name="t1")
                        t4 = rp_pool.tile([128, 256], F32, tag="rp", name="t4")
                        nc.vector.tensor_mul(t1, ps_a, cos_t[:, csl])
                        nc.vector.tensor_mul(t4, ps_a, sin_t[:, csl])
                        t2 = rp_pool.tile([128, 256], F32, tag="rp", name="t2")
                        t3 = rp_pool.tile([128, 256], F32, tag="rp", name="t3")
                        nc.vector.tensor_mul(t2, ps_b, sin_t[:, csl])
                        nc.vector.tensor_mul(t3, ps_b, cos_t[:, csl])
                        nc.vector.tensor_sub(d1, t1, t2)
                        nc.vector.tensor_add(d2, t3, t4)
                    if half == 0 and tt > 0:
                        emit_norm_stage2(tt - 1)
                    if jit0 and half == 0:
                        nc.sync.dma_start(wv_sb, wvT_r[:, :, :])
                        nc.scalar.dma_start(hids[1], hidT_r[:, 1, :, :])
                    if half == 1:
                        if jit0:
                            # the prologue below multiplies by the boundary
                            # masks - their load must be emitted first
                            nc.scalar.dma_start(mask_sb, masks_r[:, :, :])
                        # pre-emit block tt's first two score batches: q RoPE
                        # is complete; their ACT chain overlaps the V matmuls
                        bst = make_B_state(tt, q_t)
                        bst["scores"](0)
                        bst["scores"](1)
                        bst["emitted"] = 2
                    for j in range(2):
                        ps_v = psO.tile([128, HD], F32, tag="po", name="ps_v")
                        for ko in range(KO):
                            nc.tensor.matmul(
                                ps_v,
                                hid_h[:, ko, bass.ts(j, 128)],
                                wv_sb[:, ko, :],
                                start=(ko == 0), stop=(ko == KO - 1),
                            )
                        nc.scalar.copy(v_sb[tt][:, half * 2 + j, :], ps_v)
                return bst

            def emit_C_chunks(qb, tail=False):
                """o-proj partial for query block qb: 28 chunk generators.
                In-loop copies ride DVE (ACT owns the softmax chain); the
                final block alternates ACT/DVE since both are idle then."""
                ao_h0 = ao_store.pop((qb, 0))
                ao_h1 = ao_store.pop((qb, 1))
                aos = [ao_h0[0], ao_h0[1], ao_h1[0], ao_h1[1]]
                idx = 0
                for tsub in range(4):
                    for hc in range(HC):
                        ps = psS.tile([128, 512], F32, tag="s", name="psC")
                        for fs in range(4):
                            nc.tensor.matmul(
                                ps,
                                aos[fs][:, bass.ts(tsub, 128)],
                                wo_sb[:, fs, bass.ts(hc, 512)],
                                start=(fs == 0), stop=(fs == 3),
                                skip_group_check=True,
                            )
                        ot = out_pool.tile([128, 512], BF16, tag="ot",
                                           name="ot")
                        if tail and idx % 2 == 0:
                            nc.scalar.copy(ot, ps)
                        else:
                            nc.vector.tensor_scalar_add(ot, ps, 0.0)
                        r0 = qb * 512 + tsub * 128
                        nc.sync.dma_start(
                            out[r0:r0 + 128, bass.ts(hc, 512)], ot
                        )
                        idx += 1
                        yield

            def emit_B(bst, cgen):
                """Merged-head attention for one query block. Both heads
                share each key-subtile's stationary K/V operands; tanh/exp
                run once over the combined [128, 2, w] region. Boundary key
                subtiles are restricted to their live q-column range
                [lo, hi); PV accumulation relies on per-element PSUM
                has_written bits. o-proj chunk pacing finishes two
                iterations before the block ends so the next A phase never
                waits on a chunk-copy drain."""
                qb, kts, n = bst["qb"], bst["kts"], bst["n"]
                q0 = qb * 512
                dacc, probs, scores = bst["dacc"], bst["probs"], bst["scores"]
                pos_t = [psO.tile([128, 512], F32, tag="po", name=f"po{j}")
                         for j in range(4)]

                def av(i):
                    kt = kts[i]
                    off = q0 - 128 * kt
                    lo, hi = max(0, -off), min(512, 2176 - off)
                    ttk, ksub = kt // 4, kt % 4
                    pt = probs.pop(i)
                    st, sp = (i == 0), (i == n - 1)
                    v_lo = v_sb[ttk][:, ksub, 0:128]
                    v_hi = v_sb[ttk][:, ksub, 128:256]
                    nc.tensor.matmul(pos_t[0][:, lo:hi], v_lo,
                                     pt[:, 0, lo:hi], start=st, stop=sp,
                                     skip_group_check=True)
                    nc.tensor.matmul(pos_t[2][:, lo:hi], v_lo,
                                     pt[:, 1, lo:hi], start=st, stop=sp,
                                     skip_group_check=True)
                    nc.tensor.matmul(pos_t[1][:, lo:hi], v_hi,
                                     pt[:, 0, lo:hi], start=st, stop=sp,
                                     skip_group_check=True)
                    nc.tensor.matmul(pos_t[3][:, lo:hi], v_hi,
                                     pt[:, 1, lo:hi], start=st, stop=sp,
                                     skip_group_check=True)

                LOOK = 2
                for i in range(bst["emitted"], min(LOOK, n)):
                    scores(i)
                budget = 0.0
                for i in range(n):
                    if i + LOOK < n and i + LOOK >= bst["emitted"]:
                        scores(i + LOOK)
                    av(i)
                    budget += 28.0 / max(n - 2, 1)
                    while budget >= 1.0:
                        next(cgen, None)
                        budget -= 1.0
                norm_src[(qb, 0)] = (pos_t[0], pos_t[1], dacc[:, 0, :])
                norm_src[(qb, 1)] = (pos_t[2], pos_t[3], dacc[:, 1, :])

            pre = prefetch(0)
            for tt in range(TT):
                bst = emit_A(tt, pre)
                if tt == 0:
                    # deferred low-priority loads (needed from C(0) on)
                    for fs in range(4):
                        nc.sync.dma_start(wo_sb[:, fs, :], woT_r[:, fs, :])
                if tt + 1 < TT:
                    pre = prefetch(tt + 1)
                cgen = emit_C_chunks(tt - 1) if tt > 0 else iter(())
                emit_B(bst, cgen)
                for _ in cgen:
                    pass
            emit_norm_stage1(TT - 1)
            emit_norm_stage2(TT - 1)
            for _ in emit_C_chunks(TT - 1, tail=True):
                pass

    nc.compile()
    return nc


def get_nc():
    if "nc" not in _NC_CACHE:
        _NC_CACHE["nc"] = build_nc()
    return _NC_CACHE["nc"]


def prep_in_maps(inputs):
    bf16 = ml_dtypes.bfloat16
    hs = np.asarray(inputs["hidden_states"], dtype=np.float32)
    pos = np.asarray(inputs["position_ids"]).reshape(-1).astype(np.float64)
    w_qkv = np.asarray(inputs["w_qkv"], dtype=np.float32)
    w_o = np.asarray(inputs["w_o"], dtype=np.float32)

    # hidTp[p, th, ko, q] = hs[256*th + q, 128*ko + p]
    hidTp = np.ascontiguousarray(
        hs.reshape(2 * TT, 256, KO, 128).astype(bf16).transpose(3, 0, 2, 1)
    )

    inv_freq = 1.0 / (THETA ** (np.arange(HD // 2, dtype=np.float64) * 2.0 / HD))
    ang = inv_freq[:, None] * pos[None, :]
    cosT = np.cos(ang).astype(np.float32)
    sinT = np.sin(ang).astype(np.float32)

    kk = np.arange(128)[:, None]
    qq = np.arange(512)[None, :]
    masksp = np.stack(
        [((qq - kk + o >= 0) & (qq - kk + o <= WINDOW)) for o in MASK_OFFS],
        axis=1,
    ).astype(bf16)  # [128, 8, 512]

    in_maps = []
    for c in range(N_CORES):
        wq = w_qkv[512 * c:512 * (c + 1)]
        wk = w_qkv[Q_SIZE + HD * c:Q_SIZE + HD * (c + 1)]
        wv = w_qkv[Q_SIZE + NKV * HD + HD * c:Q_SIZE + NKV * HD + HD * (c + 1)]
        # [p, pr, ko, f2] = W[256*pr + f2, 128*ko + p]
        wqk = np.concatenate([wq, wk], 0)  # [768, HID]
        wqkTp = np.ascontiguousarray(
            wqk.reshape(3, 256, KO, 128).astype(bf16).transpose(3, 0, 2, 1))
        wvTp = np.ascontiguousarray(
            wv.reshape(HD, KO, 128).astype(bf16).transpose(2, 1, 0))
        # [p, fs, h] = w_o[h, 512*c + 128*fs + p]
        woTp = np.ascontiguousarray(
            w_o[:, 512 * c:512 * (c + 1)].T
            .reshape(4, 128, HID).astype(bf16).transpose(1, 0, 2))
        in_maps.append(
            dict(hidTp=hidTp, wqkTp=wqkTp, wvTp=wvTp, woTp=woTp,
                 cosT=cosT, sinT=sinT, masksp=masksp)
        )
    return in_maps


def run(inputs, **kwargs):
    nc = get_nc()
    in_maps = prep_in_maps(inputs)
    return run_bass_kernel_spmd(nc, in_maps, list(range(N_CORES)), **kwargs)


def gather_results(res):
    """Sum the 8 full-shape bf16 partials (unshard of sum-sharded output)."""
    acc = np.zeros((S, HID), dtype=np.float64)
    for c in range(N_CORES):
        acc += np.asarray(res.results[c]["out"], dtype=np.float64)
    return acc.astype(np.float32).reshape(1, S, HID)


def kernel(**inputs):
    res = run(inputs)
    return gather_results(res)


# revision 25
# speedup vs baseline: 1.0007x; 1.0007x over previous
"""Gemma2 sliding-window attention (B=1, S=4096, HID=3584, 16 Q heads / 8 KV heads,
HD=256, window 2047, tanh softcap 50) on 8 Trainium2 NeuronCores.

Sharding: tensor-parallel over heads with NO on-device collectives. Core c owns
Q heads (2c, 2c+1) and KV head c, and computes a full-shape PARTIAL of the
output projection restricted to its own 512 attention features:
    partial_c = attn[:, 512c:512c+512] @ w_o[:, 512c:512c+512].T   [S, HID]
The host sums the 8 bf16 partials in float64 (unshard of the sum-sharded
output). This removes the AllGather + serial o-proj tail.

Per-core fused pipeline over 512-token tiles tt=0..7:
  A(tt): QKV projection (transposed for Q/K, straight for V) + NeoX RoPE.
         The normalization chain of block tt-1 (denominator matmuls -> fast
         approximate reciprocals -> partition broadcasts -> ao multiplies) is
         injected between A's projection groups, where the tensor queue is
         deep and the DVE dacc chain has long drained - it never stalls
         anything.
  B(tt): sliding-window attention for query block tt with BOTH heads merged
         per key-subtile: one combined [128, 2, 512] PSUM score tile (the
         head axis is the bank boundary), ONE batched tanh and ONE batched
         exp over both heads (amortizes the 352-cycle ACT instruction
         overhead - ACT was the B-phase bottleneck engine), shared K/V
         stationary operands, and per-element-has_written column-restricted
         boundary tiles (saves ~15% of score/PV/tanh/exp work). The
         denominator accumulates on DVE in bf16 (2x mode). o-proj chunks of
         block tt-1 interleave into the loop; their PSUM->SBUF copies ride
         the DVE (ACT stays reserved for the softmax chain).

Pipelining details:
  - Block tt's first two score batches are pre-emitted into A(tt)'s tail so
    their tanh/exp chain hides under the V-projection matmuls (no block-entry
    ACT bubble).
  - V-projection PSUMs ride the psO ring (free during A after the previous
    block's normalization reads), keeping the psS ring exclusively for the
    projection/score/o-proj rotation.
  - RoPE multiplies are ordered so each projection PSUM buffer is released
    after 2 DVE ops instead of 5.
  - o-proj chunk pacing finishes two iterations before each block ends so
    the next A phase never waits on a chunk-copy drain.
Startup: tile-0 DMAs are issued in just-in-time consumption order (hid/wqk
ko-chunked and interleaved on the sync ring; cos/sin/masks/hid-half1 on the
scalar HWDGE ring) so the first matmul starts ~4us in. The final o-proj
block alternates its copies across ACT and DVE (both idle by then) to
shorten the tail.

Measured on hardware: ~848 us (baseline of this session: 994 us), rel err
7.3e-3 vs the f64 reference; tensor engine ~94% occupied.
"""

import sys

if "/opt/trn_rl_repo" not in sys.path:
    sys.path.insert(0, "/opt/trn_rl_repo")

import numpy as np
import ml_dtypes

import concourse.bass as bass
import concourse.tile as tile
from concourse import bacc, mybir
from concourse.bass_utils import run_bass_kernel_spmd

# Problem constants (hardcoded per harness contract)
S = 4096
HID = 3584
NH, NKV, HD = 16, 8, 256
Q_SIZE = NH * HD          # 4096
SCALE = 256.0 ** -0.5     # 1/16
SOFTCAP = 50.0
WINDOW = 2048 - 1         # 2047
THETA = 10000.0

N_CORES = 8
KO = HID // 128           # 28 contraction subtiles for projections
TT = S // 512             # 8 token tiles of 512
HC = HID // 512           # 7 output-column chunks of 512
F32 = mybir.dt.float32
BF16 = mybir.dt.bfloat16

# Boundary-tile diagonal offsets (q0 - 128*kt). Interior iff 128 <= off <= 1536.
MASK_OFFS = [-384, -256, -128, 0, 1664, 1792, 1920, 2048]

_NC_CACHE = {}


def build_nc():
    nc = bacc.Bacc()

    # All inputs are host-side pre-arranged to [128 partitions, ...contiguous]
    # so every DMA is 128 large contiguous descriptors.
    hidT_r = nc.declare_dram_parameter("hidTp", [128, 2 * TT, KO, 256], BF16,
                                       isOutput=False)
    wqkT_r = nc.declare_dram_parameter("wqkTp", [128, 3, KO, 256], BF16,
                                       isOutput=False)
    wvT_r = nc.declare_dram_parameter("wvTp", [128, KO, HD], BF16,
                                      isOutput=False)
    woT_r = nc.declare_dram_parameter("woTp", [128, 4, HID], BF16,
                                      isOutput=False)
    cosT = nc.declare_dram_parameter("cosT", [128, S], F32, isOutput=False)
    sinT = nc.declare_dram_parameter("sinT", [128, S], F32, isOutput=False)
    masks_r = nc.declare_dram_parameter("masksp", [128, 8, 512], BF16,
                                        isOutput=False)
    out = nc.declare_dram_parameter("out", [S, HID], BF16, isOutput=True)

    with tile.TileContext(nc) as tc:
        with (
            tc.tile_pool(name="persist", bufs=1) as persist,
            tc.tile_pool(name="hidp", bufs=2) as hid_pool,
            tc.tile_pool(name="cs", bufs=2) as cs_pool,
            tc.tile_pool(name="qp", bufs=2) as q_pool,
            tc.tile_pool(name="rp", bufs=4) as rp_pool,
            tc.tile_pool(name="probs", bufs=4) as probs_pool,
            tc.tile_pool(name="aop", bufs=8) as ao_pool,
            tc.tile_pool(name="otp", bufs=5) as out_pool,
            tc.tile_pool(name="small", bufs=2) as small_pool,
            tc.tile_pool(name="dap", bufs=2) as dacc_pool,
            tc.tile_pool(name="psS", bufs=2, space="PSUM") as psS,
            tc.tile_pool(name="psO", bufs=4, space="PSUM") as psO,
        ):
            # Persistent SBUF: weights, per-tile K/V, masks, ones.
            wqk_sb = persist.tile([128, 3, KO, 256], BF16, tag="wqk")
            wv_sb = persist.tile([128, KO, HD], BF16, tag="wv")
            mask_sb = persist.tile([128, 8, 512], BF16, tag="mask")
            wo_sb = persist.tile([128, 4, HID], BF16, tag="wo")
            ones_bf = persist.tile([128, 1], BF16, tag="ones")
            nc.vector.memset(ones_bf, 1.0)
            # HAM warm-up: dependency-free dummy matmuls keep the PE busy
            # through the startup DMA waits so the real projections run at
            # the unthrottled 2.4 GHz clock from the start
            warm_sb = persist.tile([128, 512], BF16, tag="warm")
            nc.vector.memset(warm_sb, 0.0)
            warm_ps = psO.tile([128, 512], F32, tag="po", name="warm")
            for _ in range(16):
                nc.tensor.matmul(warm_ps, warm_sb[:, 0:128], warm_sb,
                                 start=True, stop=True,
                                 skip_group_check=True)

            k_sb = [persist.tile([128, 2, 512], BF16, tag=f"k{t}", name=f"k{t}")
                    for t in range(TT)]
            v_sb = [persist.tile([128, 4, HD], BF16, tag=f"v{t}", name=f"v{t}")
                    for t in range(TT)]

            ao_store = {}
            norm_src = {}
            norm_cell = {}

            def emit_norm_stage1(qb):
                """Denominator matmul + fast reciprocal + broadcast for both
                heads of block qb. Called mid-A(qb+1): the tensor queue is
                deep with projection matmuls, so the pden matmuls never
                head-of-line block, and the dacc chains drained long ago."""
                for h in (0, 1):
                    po_lo, po_hi, dacc_h = norm_src[(qb, h)]
                    pden = psS.tile([1, 512], F32, tag="s", name="pden")
                    nc.tensor.matmul(pden, ones_bf, dacc_h,
                                     start=True, stop=True,
                                     skip_group_check=True)
                    recip = small_pool.tile([1, 512], F32, tag="recip",
                                            name="recip")
                    nc.vector.reciprocal_approx_fast(recip, pden)
                    rb = small_pool.tile([128, 512], F32, tag="rb", name="rb")
                    nc.gpsimd.partition_broadcast(rb, recip)
                    norm_cell[h] = rb

            def emit_norm_stage2(qb):
                for h in (0, 1):
                    po_lo, po_hi, _ = norm_src.pop((qb, h))
                    rb = norm_cell.pop(h)
                    ao0 = ao_pool.tile([128, 512], BF16, tag="ao", name="ao0")
                    ao1 = ao_pool.tile([128, 512], BF16, tag="ao", name="ao1")
                    nc.vector.tensor_mul(ao0, po_lo, rb)
                    nc.vector.tensor_mul(ao1, po_hi, rb)
                    ao_store[(qb, h)] = (ao0, ao1)

            def prefetch(tt):
                """Issue hid/cos/sin DMAs for tile tt. Tile 0 gets a
                just-in-time order: hid/wqk ko-chunked and interleaved so the
                first matmul group is gated on ~1.4 MB, not 6 MB."""
                hids = []
                if tt == 0:
                    hid_h0 = hid_pool.tile([128, KO, 256], BF16, tag="hid",
                                           name="hid_h")
                    # fine-grained ko-interleave: each 7-ko chunk of hid and
                    # wqk pair 0 lands just before the projection consumes it
                    for c0 in range(0, KO, 7):
                        c1 = min(c0 + 7, KO)
                        nc.sync.dma_start(hid_h0[:, c0:c1, :],
                                          hidT_r[:, 0, c0:c1, :])
                        nc.sync.dma_start(wqk_sb[:, 0, c0:c1, :],
                                          wqkT_r[:, 0, c0:c1, :])
                    # pairs 1+2 in one large transfer: 28KB/partition
                    # descriptors run ~15% closer to HBM line rate
                    nc.sync.dma_start(wqk_sb[:, 1:3, :, :],
                                      wqkT_r[:, 1:3, :, :])
                    hids.append(hid_h0)
                    # small/late tile-0 transfers ride the second HWDGE
                    # ring (scalar): the sync-ring weight stream lands sooner
                    cos_t = cs_pool.tile([128, 512], F32, tag="cos",
                                         name="cos_t")
                    nc.scalar.dma_start(cos_t, cosT[:, bass.ts(tt, 512)])
                    sin_t = cs_pool.tile([128, 512], F32, tag="sin",
                                         name="sin_t")
                    nc.scalar.dma_start(sin_t, sinT[:, bass.ts(tt, 512)])
                    # second hid half + remaining weights are emitted JIT
                    # inside emit_A(0)
                    hid_h1 = hid_pool.tile([128, KO, 256], BF16, tag="hid",
                                           name="hid_h")
                    hids.append(hid_h1)
                    return hids, cos_t, sin_t, True
                for half in range(2):
                    hid_h = hid_pool.tile([128, KO, 256], BF16, tag="hid",
                                          name="hid_h")
                    nc.sync.dma_start(hid_h, hidT_r[:, tt * 2 + half, :, :])
                    hids.append(hid_h)
                cos_t = cs_pool.tile([128, 512], F32, tag="cos", name="cos_t")
                nc.sync.dma_start(cos_t, cosT[:, bass.ts(tt, 512)])
                sin_t = cs_pool.tile([128, 512], F32, tag="sin", name="sin_t")
                nc.sync.dma_start(sin_t, sinT[:, bass.ts(tt, 512)])
                return hids, cos_t, sin_t, False

            def make_B_state(qb, q_t):
                """Closures + state for block qb's merged-head attention.
                scores() is usable as soon as q RoPE is done, so the first
                two batches can be pre-emitted into A's tail (their tanh/exp
                hide under the V-projection matmuls, removing the block-entry
                ACT bubble)."""
                q0 = qb * 512
                kts = list(range(max(0, 4 * qb - 16), 4 * qb + 4))
                n = len(kts)
                dacc = dacc_pool.tile([128, 2, 512], BF16, tag="da",
                                      name="dacc")
                nc.vector.memset(dacc, 0.0)
                probs = {}
                st = dict(qb=qb, kts=kts, n=n, dacc=dacc, probs=probs,
                          emitted=0, pos_t=None)

                def scores(i):
                    kt = kts[i]
                    off = q0 - 128 * kt
                    lo, hi = max(0, -off), min(512, 2176 - off)
                    ttk, ksub = kt // 4, kt % 4
                    ksl = bass.ts(ksub, 128)
                    ps = psS.tile([128, 2, 512], F32, tag="s", name="ps_s")
                    nc.tensor.matmul(ps[:, 0, lo:hi], k_sb[ttk][:, 0, ksl],
                                     q_t[:, 0, lo:hi], start=True, stop=False)
                    nc.tensor.matmul(ps[:, 1, lo:hi], k_sb[ttk][:, 0, ksl],
                                     q_t[:, 2, lo:hi], start=True, stop=False)
                    nc.tensor.matmul(ps[:, 0, lo:hi], k_sb[ttk][:, 1, ksl],
                                     q_t[:, 1, lo:hi], start=False, stop=True)
                    nc.tensor.matmul(ps[:, 1, lo:hi], k_sb[ttk][:, 1, ksl],
                                     q_t[:, 3, lo:hi], start=False, stop=True)
                    pt = probs_pool.tile([128, 2, 512], BF16, tag="pt",
                                         name="pt")
                    nc.scalar.activation(
                        ps[:, :, lo:hi], ps[:, :, lo:hi],
                        mybir.ActivationFunctionType.Tanh,
                        scale=SCALE / SOFTCAP,
                    )
                    nc.scalar.activation(
                        pt[:, :, lo:hi], ps[:, :, lo:hi],
                        mybir.ActivationFunctionType.Exp,
                        scale=SOFTCAP,
                    )
                    if not (128 <= off <= 1536):
                        mi = MASK_OFFS.index(off)
                        for hh in (0, 1):
                            nc.vector.tensor_mul(pt[:, hh, lo:hi],
                                                 pt[:, hh, lo:hi],
                                                 mask_sb[:, mi, lo:hi])
                    if lo == 0 and hi == 512:
                        nc.vector.tensor_add(dacc, dacc, pt)
                    else:
                        for hh in (0, 1):
                            nc.vector.tensor_add(dacc[:, hh, lo:hi],
                                                 dacc[:, hh, lo:hi],
                                                 pt[:, hh, lo:hi])
                    probs[i] = pt

                st["scores"] = scores
                return st

            def emit_A(tt, pre):
                """QKV projection + RoPE for token tile tt. Returns the B
                state for block tt. Block tt-1's normalization stages are
                injected between projection groups of the first half; block
                tt's first two score batches are pre-emitted before the
                second half's V-projection groups."""
                hids, cos_t, sin_t, jit0 = pre
                q_t = q_pool.tile([128, 4, 512], BF16, tag="q", name="q_t")
                bst = None
                for half in range(2):
                    csl = bass.ts(half, 256)
                    hid_h = hids[half]
                    for pair in range(3):
                        if jit0 and half == 0 and pair == 1:
                            # fill the pair0->pair1 weight-DMA wait
                            for _ in range(20):
                                nc.tensor.matmul(warm_ps, warm_sb[:, 0:128],
                                                 warm_sb, start=True,
                                                 stop=True,
                                                 skip_group_check=True)
                        if half == 0 and pair == 2 and tt > 0:
                            emit_norm_stage1(tt - 1)
                        ps_a = psS.tile([128, 256], F32, tag="s", name="ps_a")
                        for ko in range(KO):
                            nc.tensor.matmul(
                                ps_a,
                                wqk_sb[:, pair, ko, 0:128],
                                hid_h[:, ko, :],
                                start=(ko == 0), stop=(ko == KO - 1),
                            )
                        ps_b = psS.tile([128, 256], F32, tag="s", name="ps_b")
                        for ko in range(KO):
                            nc.tensor.matmul(
                                ps_b,
                                wqk_sb[:, pair, ko, 128:256],
                                hid_h[:, ko, :],
                                start=(ko == 0), stop=(ko == KO - 1),
                            )
                        if pair < 2:
                            d1 = q_t[:, 2 * pair, csl]
                            d2 = q_t[:, 2 * pair + 1, csl]
                        else:
                            d1 = k_sb[tt][:, 0, csl]
                            d2 = k_sb[tt][:, 1, csl]
                        # ps_a is read by the first two DVE ops and ps_b by
                        # the next two, so each PSUM ring buffer frees after
                        # 2 ops instead of 5 - the next projection group's
                        # matmuls unblock ~1.5us earlier
                        t1 = rp_pool.tile([128, 256], F32, tag="rp", name="t1")
                        t4 = rp_pool.tile([128, 256], F32, tag="rp", name="t4")
                        nc.vector.tensor_mul(t1, ps_a, cos_t[:, csl])
                        nc.vector.tensor_mul(t4, ps_a, sin_t[:, csl])
                        t2 = rp_pool.tile([128, 256], F32, tag="rp", name="t2")
                        t3 = rp_pool.tile([128, 256], F32, tag="rp", name="t3")
                        nc.vector.tensor_mul(t2, ps_b, sin_t[:, csl])
                        nc.vector.tensor_mul(t3, ps_b, cos_t[:, csl])
                        nc.vector.tensor_sub(d1, t1, t2)
                        nc.vector.tensor_add(d2, t3, t4)
                    if half == 0 and tt > 0:
                        emit_norm_stage2(tt - 1)
                    if jit0 and half == 0:
                        nc.sync.dma_start(wv_sb, wvT_r[:, :, :])
                        nc.scalar.dma_start(hids[1], hidT_r[:, 1, :, :])
                    if half == 1:
                        if jit0:
                            # the prologue below multiplies by the boundary
                            # masks - their load must be emitted first
                            nc.scalar.dma_start(mask_sb, masks_r[:, :, :])
                        # pre-emit block tt's first two score batches: q RoPE
                        # is complete; their ACT chain overlaps the V matmuls
                        bst = make_B_state(tt, q_t)
                        bst["scores"](0)
                        bst["scores"](1)
                        bst["emitted"] = 2
                    for j in range(2):
                        ps_v = psO.tile([128, HD], F32, tag="po", name="ps_v")
                        for ko in range(KO):
                            nc.tensor.matmul(
                                ps_v,
                                hid_h[:, ko, bass.ts(j, 128)],
                                wv_sb[:, ko, :],
                                start=(ko == 0), stop=(ko == KO - 1),
                            )
                        nc.scalar.copy(v_sb[tt][:, half * 2 + j, :], ps_v)
                return bst

            def emit_C_chunks(qb, tail=False):
                """o-proj partial for query block qb: 28 chunk generators.
                In-loop copies ride DVE (ACT owns the softmax chain); the
                final block alternates ACT/DVE since both are idle then."""
                ao_h0 = ao_store.pop((qb, 0))
                ao_h1 = ao_store.pop((qb, 1))
                aos = [ao_h0[0], ao_h0[1], ao_h1[0], ao_h1[1]]
                idx = 0
                for tsub in range(4):
                    for hc in range(HC):
                        ps = psS.tile([128, 512], F32, tag="s", name="psC")
                        for fs in range(4):
                            nc.tensor.matmul(
                                ps,
                                aos[fs][:, bass.ts(tsub, 128)],
                                wo_sb[:, fs, bass.ts(hc, 512)],
                                start=(fs == 0), stop=(fs == 3),
                                skip_group_check=True,
                            )
                        ot = out_pool.tile([128, 512], BF16, tag="ot",
                                           name="ot")
                        if tail and idx % 2 == 0:
                            nc.scalar.copy(ot, ps)
                        else:
                            nc.vector.tensor_scalar_add(ot, ps, 0.0)
                        r0 = qb * 512 + tsub * 128
                        nc.sync.dma_start(
                            out[r0:r0 + 128, bass.ts(hc, 512)], ot
                        )
                        idx += 1
                        yield

            def emit_B(bst, cgen, last=False):
                """Merged-head attention for one query block. Both heads
                share each key-subtile's stationary K/V operands; tanh/exp
                run once over the combined [128, 2, w] region. Boundary key
                subtiles are restricted to their live q-column range
                [lo, hi); PV accumulation relies on per-element PSUM
                has_written bits. o-proj chunk pacing finishes two
                iterations before the block ends so the next A phase never
                waits on a chunk-copy drain."""
                qb, kts, n = bst["qb"], bst["kts"], bst["n"]
                q0 = qb * 512
                dacc, probs, scores = bst["dacc"], bst["probs"], bst["scores"]
                pos_t = [psO.tile([128, 512], F32, tag="po", name=f"po{j}")
                         for j in range(4)]

                def av(i):
                    kt = kts[i]
                    off = q0 - 128 * kt
                    lo, hi = max(0, -off), min(512, 2176 - off)
                    ttk, ksub = kt // 4, kt % 4
                    pt = probs.pop(i)
                    st, sp = (i == 0), (i == n - 1)
                    v_lo = v_sb[ttk][:, ksub, 0:128]
                    v_hi = v_sb[ttk][:, ksub, 128:256]
                    nc.tensor.matmul(pos_t[0][:, lo:hi], v_lo,
                                     pt[:, 0, lo:hi], start=st, stop=sp,
                                     skip_group_check=True)
                    nc.tensor.matmul(pos_t[2][:, lo:hi], v_lo,
                                     pt[:, 1, lo:hi], start=st, stop=sp,
                                     skip_group_check=True)
                    nc.tensor.matmul(pos_t[1][:, lo:hi], v_hi,
                                     pt[:, 0, lo:hi], start=st, stop=sp,
                                     skip_group_check=True)
                    nc.tensor.matmul(pos_t[3][:, lo:hi], v_hi,
                                     pt[:, 1, lo:hi], start=st, stop=sp,
                                     skip_group_check=True)

                LOOK = 2
                for i in range(bst["emitted"], min(LOOK, n)):
                    scores(i)
                budget = 0.0
                for i in range(n):
                    if i + LOOK < n and i + LOOK >= bst["emitted"]:
                        scores(i + LOOK)
                    av(i)
                    budget += 28.0 / (n if last else max(n - 2, 1))
                    while budget >= 1.0:
                        next(cgen, None)
                        budget -= 1.0
                norm_src[(qb, 0)] = (pos_t[0], pos_t[1], dacc[:, 0, :])
                norm_src[(qb, 1)] = (pos_t[2], pos_t[3], dacc[:, 1, :])

            pre = prefetch(0)
            for tt in range(TT):
                bst = emit_A(tt, pre)
                if tt == 0:
                    # deferred low-priority loads (needed from C(0) on)
                    for fs in range(4):
                        nc.sync.dma_start(wo_sb[:, fs, :], woT_r[:, fs, :])
                if tt + 1 < TT:
                    pre = prefetch(tt + 1)
                cgen = emit_C_chunks(tt - 1) if tt > 0 else iter(())
                emit_B(bst, cgen, last=(tt == TT - 1))
                for _ in cgen:
                    pass
            emit_norm_stage1(TT - 1)
            emit_norm_stage2(TT - 1)
            for _ in emit_C_chunks(TT - 1, tail=True):
                pass

    nc.compile()
    return nc


def get_nc():
    if "nc" not in _NC_CACHE:
        _NC_CACHE["nc"] = build_nc()
    return _NC_CACHE["nc"]


def prep_in_maps(inputs):
    bf16 = ml_dtypes.bfloat16
    hs = np.asarray(inputs["hidden_states"], dtype=np.float32)
    pos = np.asarray(inputs["position_ids"]).reshape(-1).astype(np.float64)
    w_qkv = np.asarray(inputs["w_qkv"], dtype=np.float32)
    w_o = np.asarray(inputs["w_o"], dtype=np.float32)

    # hidTp[p, th, ko, q] = hs[256*th + q, 128*ko + p]
    hidTp = np.ascontiguousarray(
        hs.reshape(2 * TT, 256, KO, 128).astype(bf16).transpose(3, 0, 2, 1)
    )

    inv_freq = 1.0 / (THETA ** (np.arange(HD // 2, dtype=np.float64) * 2.0 / HD))
    ang = inv_freq[:, None] * pos[None, :]
    cosT = np.cos(ang).astype(np.float32)
    sinT = np.sin(ang).astype(np.float32)

    kk = np.arange(128)[:, None]
    qq = np.arange(512)[None, :]
    masksp = np.stack(
        [((qq - kk + o >= 0) & (qq - kk + o <= WINDOW)) for o in MASK_OFFS],
        axis=1,
    ).astype(bf16)  # [128, 8, 512]

    in_maps = []
    for c in range(N_CORES):
        wq = w_qkv[512 * c:512 * (c + 1)]
        wk = w_qkv[Q_SIZE + HD * c:Q_SIZE + HD * (c + 1)]
        wv = w_qkv[Q_SIZE + NKV * HD + HD * c:Q_SIZE + NKV * HD + HD * (c + 1)]
        # [p, pr, ko, f2] = W[256*pr + f2, 128*ko + p]
        wqk = np.concatenate([wq, wk], 0)  # [768, HID]
        wqkTp = np.ascontiguousarray(
            wqk.reshape(3, 256, KO, 128).astype(bf16).transpose(3, 0, 2, 1))
        wvTp = np.ascontiguousarray(
            wv.reshape(HD, KO, 128).astype(bf16).transpose(2, 1, 0))
        # [p, fs, h] = w_o[h, 512*c + 128*fs + p]
        woTp = np.ascontiguousarray(
            w_o[:, 512 * c:512 * (c + 1)].T
            .reshape(4, 128, HID).astype(bf16).transpose(1, 0, 2))
        in_maps.append(
            dict(hidTp=hidTp, wqkTp=wqkTp, wvTp=wvTp, woTp=woTp,
                 cosT=cosT, sinT=sinT, masksp=masksp)
        )
    return in_maps


def run(inputs, **kwargs):
    nc = get_nc()
    in_maps = prep_in_maps(inputs)
    return run_bass_kernel_spmd(nc, in_maps, list(range(N_CORES)), **kwargs)


def gather_results(res):
    """Sum the 8 full-shape bf16 partials (unshard of sum-sharded output)."""
    acc = np.zeros((S, HID), dtype=np.float64)
    for c in range(N_CORES):
        acc += np.asarray(res.results[c]["out"], dtype=np.float64)
    return acc.astype(np.float32).reshape(1, S, HID)


def kernel(**inputs):
    res = run(inputs)
    return gather_results(res)


# revision 26
# speedup vs baseline: 1.0086x; 1.0080x over previous
"""Gemma2 sliding-window attention (B=1, S=4096, HID=3584, 16 Q heads / 8 KV heads,
HD=256, window 2047, tanh softcap 50) on 8 Trainium2 NeuronCores.

Sharding: tensor-parallel over heads with NO on-device collectives. Core c owns
Q heads (2c, 2c+1) and KV head c, and computes a full-shape PARTIAL of the
output projection restricted to its own 512 attention features:
    partial_c = attn[:, 512c:512c+512] @ w_o[:, 512c:512c+512].T   [S, HID]
The host sums the 8 bf16 partials in float64 (unshard of the sum-sharded
output). This removes the AllGather + serial o-proj tail.

Per-core fused pipeline over 512-token tiles tt=0..7:
  A(tt): QKV projection (transposed for Q/K, straight for V) + NeoX RoPE.
         The normalization chain of block tt-1 (denominator matmuls -> fast
         approximate reciprocals -> partition broadcasts -> ao multiplies) is
         injected between A's projection groups, where the tensor queue is
         deep and the DVE dacc chain has long drained - it never stalls
         anything.
  B(tt): sliding-window attention for query block tt with BOTH heads merged
         per key-subtile: one combined [128, 2, 512] PSUM score tile (the
         head axis is the bank boundary), ONE batched tanh and ONE batched
         exp over both heads (amortizes the 352-cycle ACT instruction
         overhead - ACT was the B-phase bottleneck engine), shared K/V
         stationary operands, and per-element-has_written column-restricted
         boundary tiles (saves ~15% of score/PV/tanh/exp work). The
         denominator accumulates on DVE in bf16 (2x mode). o-proj chunks of
         block tt-1 interleave into the loop; their PSUM->SBUF copies ride
         the DVE (ACT stays reserved for the softmax chain).

Pipelining details:
  - Block tt's first two score batches are pre-emitted into A(tt)'s tail so
    their tanh/exp chain hides under the V-projection matmuls (no block-entry
    ACT bubble).
  - V-projection PSUMs ride the psO ring (free during A after the previous
    block's normalization reads), keeping the psS ring exclusively for the
    projection/score/o-proj rotation.
  - RoPE multiplies are ordered so each projection PSUM buffer is released
    after 2 DVE ops instead of 5.
  - o-proj chunk pacing finishes two iterations before each block ends so
    the next A phase never waits on a chunk-copy drain.
Startup: tile-0 DMAs are issued in just-in-time consumption order (hid/wqk
ko-chunked and interleaved on the sync ring; cos/sin/masks/hid-half1 on the
scalar HWDGE ring) so the first matmul starts ~4us in. The final o-proj
block alternates its copies across ACT and DVE (both idle by then) to
shorten the tail.

Measured on hardware: ~848 us (baseline of this session: 994 us), rel err
7.3e-3 vs the f64 reference; tensor engine ~94% occupied.
"""

import sys

if "/opt/trn_rl_repo" not in sys.path:
    sys.path.insert(0, "/opt/trn_rl_repo")

import numpy as np
import ml_dtypes

import concourse.bass as bass
import concourse.tile as tile
from concourse import bacc, mybir
from concourse.bass_utils import run_bass_kernel_spmd

# Problem constants (hardcoded per harness contract)
S = 4096
HID = 3584
NH, NKV, HD = 16, 8, 256
Q_SIZE = NH * HD          # 4096
SCALE = 256.0 ** -0.5     # 1/16
SOFTCAP = 50.0
WINDOW = 2048 - 1         # 2047
THETA = 10000.0

N_CORES = 8
KO = HID // 128           # 28 contraction subtiles for projections
TT = S // 512             # 8 token tiles of 512
HC = HID // 512           # 7 output-column chunks of 512
F32 = mybir.dt.float32
BF16 = mybir.dt.bfloat16

# Boundary-tile diagonal offsets (q0 - 128*kt). Interior iff 128 <= off <= 1536.
MASK_OFFS = [-384, -256, -128, 0, 1664, 1792, 1920, 2048]

_NC_CACHE = {}


def build_nc():
    nc = bacc.Bacc()

    # All inputs are host-side pre-arranged to [128 partitions, ...contiguous]
    # so every DMA is 128 large contiguous descriptors.
    hidT_r = nc.declare_dram_parameter("hidTp", [128, 2 * TT, KO, 256], BF16,
                                       isOutput=False)
    wqkT_r = nc.declare_dram_parameter("wqkTp", [128, 3, KO, 256], BF16,
                                       isOutput=False)
    wvT_r = nc.declare_dram_parameter("wvTp", [128, KO, HD], BF16,
                                      isOutput=False)
    woT_r = nc.declare_dram_parameter("woTp", [128, 4, HID], BF16,
                                      isOutput=False)
    cosT = nc.declare_dram_parameter("cosT", [128, S], F32, isOutput=False)
    sinT = nc.declare_dram_parameter("sinT", [128, S], F32, isOutput=False)
    masks_r = nc.declare_dram_parameter("masksp", [128, 8, 512], BF16,
                                        isOutput=False)
    out = nc.declare_dram_parameter("out", [S, HID], BF16, isOutput=True)

    with tile.TileContext(nc) as tc:
        with (
            tc.tile_pool(name="persist", bufs=1) as persist,
            tc.tile_pool(name="hidp", bufs=2) as hid_pool,
            tc.tile_pool(name="cs", bufs=2) as cs_pool,
            tc.tile_pool(name="qp", bufs=2) as q_pool,
            tc.tile_pool(name="rp", bufs=4) as rp_pool,
            tc.tile_pool(name="probs", bufs=4) as probs_pool,
            tc.tile_pool(name="aop", bufs=8) as ao_pool,
            tc.tile_pool(name="otp", bufs=5) as out_pool,
            tc.tile_pool(name="small", bufs=2) as small_pool,
            tc.tile_pool(name="dap", bufs=2) as dacc_pool,
            tc.tile_pool(name="psS", bufs=2, space="PSUM") as psS,
            tc.tile_pool(name="psO", bufs=4, space="PSUM") as psO,
        ):
            # Persistent SBUF: weights, per-tile K/V, masks, ones.
            wqk_sb = persist.tile([128, 3, KO, 256], BF16, tag="wqk")
            wv_sb = persist.tile([128, KO, HD], BF16, tag="wv")
            mask_sb = persist.tile([128, 8, 512], BF16, tag="mask")
            wo_sb = persist.tile([128, 4, HID], BF16, tag="wo")
            ones_bf = persist.tile([128, 1], BF16, tag="ones")
            nc.vector.memset(ones_bf, 1.0)
            # HAM warm-up: dependency-free dummy matmuls keep the PE busy
            # through the startup DMA waits so the real projections run at
            # the unthrottled 2.4 GHz clock from the start
            warm_sb = persist.tile([128, 512], BF16, tag="warm")
            nc.vector.memset(warm_sb, 0.0)
            warm_ps = psO.tile([128, 512], F32, tag="po", name="warm")
            for _ in range(16):
                nc.tensor.matmul(warm_ps, warm_sb[:, 0:128], warm_sb,
                                 start=True, stop=True,
                                 skip_group_check=True)

            k_sb = [persist.tile([128, 2, 512], BF16, tag=f"k{t}", name=f"k{t}")
                    for t in range(TT)]
            v_sb = [persist.tile([128, 4, HD], BF16, tag=f"v{t}", name=f"v{t}")
                    for t in range(TT)]

            ao_store = {}
            norm_src = {}
            norm_cell = {}

            def emit_norm_stage1(qb):
                """Denominator matmul + fast reciprocal + broadcast for both
                heads of block qb. Called mid-A(qb+1): the tensor queue is
                deep with projection matmuls, so the pden matmuls never
                head-of-line block, and the dacc chains drained long ago."""
                for h in (0, 1):
                    po_lo, po_hi, dacc_h = norm_src[(qb, h)]
                    pden = psS.tile([1, 512], F32, tag="s", name="pden")
                    nc.tensor.matmul(pden, ones_bf, dacc_h,
                                     start=True, stop=True,
                                     skip_group_check=True)
                    recip = small_pool.tile([1, 512], F32, tag="recip",
                                            name="recip")
                    nc.vector.reciprocal_approx_fast(recip, pden)
                    rb = small_pool.tile([128, 512], F32, tag="rb", name="rb")
                    nc.gpsimd.partition_broadcast(rb, recip)
                    norm_cell[h] = rb

            def emit_norm_stage2(qb):
                for h in (0, 1):
                    po_lo, po_hi, _ = norm_src.pop((qb, h))
                    rb = norm_cell.pop(h)
                    ao0 = ao_pool.tile([128, 512], BF16, tag="ao", name="ao0")
                    ao1 = ao_pool.tile([128, 512], BF16, tag="ao", name="ao1")
                    nc.vector.tensor_mul(ao0, po_lo, rb)
                    nc.vector.tensor_mul(ao1, po_hi, rb)
                    ao_store[(qb, h)] = (ao0, ao1)

            def prefetch(tt):
                """Issue hid/cos/sin DMAs for tile tt. Tile 0 gets a
                just-in-time order: hid/wqk ko-chunked and interleaved so the
                first matmul group is gated on ~1.4 MB, not 6 MB."""
                hids = []
                if tt == 0:
                    hid_h0 = hid_pool.tile([128, KO, 256], BF16, tag="hid",
                                           name="hid_h")
                    # fine-grained ko-interleave: each 7-ko chunk of hid and
                    # wqk pair 0 lands just before the projection consumes it
                    for c0 in range(0, KO, 7):
                        c1 = min(c0 + 7, KO)
                        nc.sync.dma_start(hid_h0[:, c0:c1, :],
                                          hidT_r[:, 0, c0:c1, :])
                        nc.sync.dma_start(wqk_sb[:, 0, c0:c1, :],
                                          wqkT_r[:, 0, c0:c1, :])
                    nc.sync.dma_start(wqk_sb[:, 1, 0:14, :],
                                      wqkT_r[:, 1, 0:14, :])
                    nc.sync.dma_start(wqk_sb[:, 1, 14:KO, :],
                                      wqkT_r[:, 1, 14:KO, :])
                    hids.append(hid_h0)
                    # small/late tile-0 transfers ride the second HWDGE
                    # ring (scalar): the sync-ring weight stream lands sooner
                    cos_t = cs_pool.tile([128, 512], F32, tag="cos",
                                         name="cos_t")
                    nc.scalar.dma_start(cos_t, cosT[:, bass.ts(tt, 512)])
                    sin_t = cs_pool.tile([128, 512], F32, tag="sin",
                                         name="sin_t")
                    nc.scalar.dma_start(sin_t, sinT[:, bass.ts(tt, 512)])
                    # second hid half + remaining weights are emitted JIT
                    # inside emit_A(0)
                    hid_h1 = hid_pool.tile([128, KO, 256], BF16, tag="hid",
                                           name="hid_h")
                    hids.append(hid_h1)
                    return hids, cos_t, sin_t, True
                for half in range(2):
                    hid_h = hid_pool.tile([128, KO, 256], BF16, tag="hid",
                                          name="hid_h")
                    nc.sync.dma_start(hid_h, hidT_r[:, tt * 2 + half, :, :])
                    hids.append(hid_h)
                cos_t = cs_pool.tile([128, 512], F32, tag="cos", name="cos_t")
                nc.sync.dma_start(cos_t, cosT[:, bass.ts(tt, 512)])
                sin_t = cs_pool.tile([128, 512], F32, tag="sin", name="sin_t")
                nc.sync.dma_start(sin_t, sinT[:, bass.ts(tt, 512)])
                return hids, cos_t, sin_t, False

            def make_B_state(qb, q_t):
                """Closures + state for block qb's merged-head attention.
                scores() is usable as soon as q RoPE is done, so the first
                two batches can be pre-emitted into A's tail (their tanh/exp
                hide under the V-projection matmuls, removing the block-entry
                ACT bubble)."""
                q0 = qb * 512
                kts = list(range(max(0, 4 * qb - 16), 4 * qb + 4))
                n = len(kts)
                dacc = dacc_pool.tile([128, 2, 512], BF16, tag="da",
                                      name="dacc")
                nc.vector.memset(dacc, 0.0)
                probs = {}
                st = dict(qb=qb, kts=kts, n=n, dacc=dacc, probs=probs,
                          emitted=0, pos_t=None)

                def scores(i):
                    kt = kts[i]
                    off = q0 - 128 * kt
                    lo, hi = max(0, -off), min(512, 2176 - off)
                    ttk, ksub = kt // 4, kt % 4
                    ksl = bass.ts(ksub, 128)
                    ps = psS.tile([128, 2, 512], F32, tag="s", name="ps_s")
                    nc.tensor.matmul(ps[:, 0, lo:hi], k_sb[ttk][:, 0, ksl],
                                     q_t[:, 0, lo:hi], start=True, stop=False)
                    nc.tensor.matmul(ps[:, 1, lo:hi], k_sb[ttk][:, 0, ksl],
                                     q_t[:, 2, lo:hi], start=True, stop=False)
                    nc.tensor.matmul(ps[:, 0, lo:hi], k_sb[ttk][:, 1, ksl],
                                     q_t[:, 1, lo:hi], start=False, stop=True)
                    nc.tensor.matmul(ps[:, 1, lo:hi], k_sb[ttk][:, 1, ksl],
                                     q_t[:, 3, lo:hi], start=False, stop=True)
                    pt = probs_pool.tile([128, 2, 512], BF16, tag="pt",
                                         name="pt")
                    nc.scalar.activation(
                        ps[:, :, lo:hi], ps[:, :, lo:hi],
                        mybir.ActivationFunctionType.Tanh,
                        scale=SCALE / SOFTCAP,
                    )
                    nc.scalar.activation(
                        pt[:, :, lo:hi], ps[:, :, lo:hi],
                        mybir.ActivationFunctionType.Exp,
                        scale=SOFTCAP,
                    )
                    if not (128 <= off <= 1536):
                        mi = MASK_OFFS.index(off)
                        for hh in (0, 1):
                            nc.vector.tensor_mul(pt[:, hh, lo:hi],
                                                 pt[:, hh, lo:hi],
                                                 mask_sb[:, mi, lo:hi])
                    if lo == 0 and hi == 512:
                        nc.vector.tensor_add(dacc, dacc, pt)
                    else:
                        for hh in (0, 1):
                            nc.vector.tensor_add(dacc[:, hh, lo:hi],
                                                 dacc[:, hh, lo:hi],
                                                 pt[:, hh, lo:hi])
                    probs[i] = pt

                st["scores"] = scores
                return st

            def emit_A(tt, pre):
                """QKV projection + RoPE for token tile tt. Returns the B
                state for block tt. Block tt-1's normalization stages are
                injected between projection groups of the first half; block
                tt's first two score batches are pre-emitted before the
                second half's V-projection groups."""
                hids, cos_t, sin_t, jit0 = pre
                q_t = q_pool.tile([128, 4, 512], BF16, tag="q", name="q_t")
                bst = None
                for half in range(2):
                    csl = bass.ts(half, 256)
                    hid_h = hids[half]
                    for pair in range(3):
                        if jit0 and half == 0 and pair == 2:
                            nc.sync.dma_start(wqk_sb[:, 2, 0:14, :],
                                              wqkT_r[:, 2, 0:14, :])
                            nc.sync.dma_start(wqk_sb[:, 2, 14:KO, :],
                                              wqkT_r[:, 2, 14:KO, :])
                        if jit0 and half == 0 and pair == 1:
                            # fill the pair0->pair1 weight-DMA wait
                            for _ in range(20):
                                nc.tensor.matmul(warm_ps, warm_sb[:, 0:128],
                                                 warm_sb, start=True,
                                                 stop=True,
                                                 skip_group_check=True)
                        if half == 0 and pair == 2 and tt > 0:
                            emit_norm_stage1(tt - 1)
                        ps_a = psS.tile([128, 256], F32, tag="s", name="ps_a")
                        for ko in range(KO):
                            nc.tensor.matmul(
                                ps_a,
                                wqk_sb[:, pair, ko, 0:128],
                                hid_h[:, ko, :],
                                start=(ko == 0), stop=(ko == KO - 1),
                            )
                        ps_b = psS.tile([128, 256], F32, tag="s", name="ps_b")
                        for ko in range(KO):
                            nc.tensor.matmul(
                                ps_b,
                                wqk_sb[:, pair, ko, 128:256],
                                hid_h[:, ko, :],
                                start=(ko == 0), stop=(ko == KO - 1),
                            )
                        if pair < 2:
                            d1 = q_t[:, 2 * pair, csl]
                            d2 = q_t[:, 2 * pair + 1, csl]
                        else:
                            d1 = k_sb[tt][:, 0, csl]
                            d2 = k_sb[tt][:, 1, csl]
                        # ps_a is read by the first two DVE ops and ps_b by
                        # the next two, so each PSUM ring buffer frees after
                        # 2 ops instead of 5 - the next projection group's
                        # matmuls unblock ~1.5us earlier
                        t1 = rp_pool.tile([128, 256], F32, tag="rp", name="t1")
                        t4 = rp_pool.tile([128, 256], F32, tag="rp", name="t4")
                        nc.vector.tensor_mul(t1, ps_a, cos_t[:, csl])
                        nc.vector.tensor_mul(t4, ps_a, sin_t[:, csl])
                        t2 = rp_pool.tile([128, 256], F32, tag="rp", name="t2")
                        t3 = rp_pool.tile([128, 256], F32, tag="rp", name="t3")
                        nc.vector.tensor_mul(t2, ps_b, sin_t[:, csl])
                        nc.vector.tensor_mul(t3, ps_b, cos_t[:, csl])
                        nc.vector.tensor_sub(d1, t1, t2)
                        nc.vector.tensor_add(d2, t3, t4)
                    if half == 0 and tt > 0:
                        emit_norm_stage2(tt - 1)
                    if jit0 and half == 0:
                        nc.sync.dma_start(wv_sb, wvT_r[:, :, :])
                        nc.scalar.dma_start(hids[1], hidT_r[:, 1, :, :])
                    if half == 1:
                        if jit0:
                            # the prologue below multiplies by the boundary
                            # masks - their load must be emitted first
                            nc.scalar.dma_start(mask_sb, masks_r[:, :, :])
                        # pre-emit block tt's first two score batches: q RoPE
                        # is complete; their ACT chain overlaps the V matmuls
                        bst = make_B_state(tt, q_t)
                        bst["scores"](0)
                        bst["scores"](1)
                        bst["emitted"] = 2
                    for j in range(2):
                        ps_v = psO.tile([128, HD], F32, tag="po", name="ps_v")
                        for ko in range(KO):
                            nc.tensor.matmul(
                                ps_v,
                                hid_h[:, ko, bass.ts(j, 128)],
                                wv_sb[:, ko, :],
                                start=(ko == 0), stop=(ko == KO - 1),
                            )
                        nc.scalar.copy(v_sb[tt][:, half * 2 + j, :], ps_v)
                return bst

            def emit_C_chunks(qb, tail=False):
                """o-proj partial for query block qb: 28 chunk generators.
                In-loop copies ride DVE (ACT owns the softmax chain); the
                final block alternates ACT/DVE since both are idle then."""
                ao_h0 = ao_store.pop((qb, 0))
                ao_h1 = ao_store.pop((qb, 1))
                aos = [ao_h0[0], ao_h0[1], ao_h1[0], ao_h1[1]]
                idx = 0
                for tsub in range(4):
                    for hc in range(HC):
                        ps = psS.tile([128, 512], F32, tag="s", name="psC")
                        for fs in range(4):
                            nc.tensor.matmul(
                                ps,
                                aos[fs][:, bass.ts(tsub, 128)],
                                wo_sb[:, fs, bass.ts(hc, 512)],
                                start=(fs == 0), stop=(fs == 3),
                                skip_group_check=True,
                            )
                        ot = out_pool.tile([128, 512], BF16, tag="ot",
                                           name="ot")
                        if tail and idx % 2 == 0:
                            nc.scalar.copy(ot, ps)
                        else:
                            nc.vector.tensor_scalar_add(ot, ps, 0.0)
                        r0 = qb * 512 + tsub * 128
                        nc.sync.dma_start(
                            out[r0:r0 + 128, bass.ts(hc, 512)], ot
                        )
                        idx += 1
                        yield

            def emit_B(bst, cgen, last=False):
                """Merged-head attention for one query block. Both heads
                share each key-subtile's stationary K/V operands; tanh/exp
                run once over the combined [128, 2, w] region. Boundary key
                subtiles are restricted to their live q-column range
                [lo, hi); PV accumulation relies on per-element PSUM
                has_written bits. o-proj chunk pacing finishes two
                iterations before the block ends so the next A phase never
                waits on a chunk-copy drain."""
                qb, kts, n = bst["qb"], bst["kts"], bst["n"]
                q0 = qb * 512
                dacc, probs, scores = bst["dacc"], bst["probs"], bst["scores"]
                pos_t = [psO.tile([128, 512], F32, tag="po", name=f"po{j}")
                         for j in range(4)]

                def av(i):
                    kt = kts[i]
                    off = q0 - 128 * kt
                    lo, hi = max(0, -off), min(512, 2176 - off)
                    ttk, ksub = kt // 4, kt % 4
                    pt = probs.pop(i)
                    st, sp = (i == 0), (i == n - 1)
                    v_lo = v_sb[ttk][:, ksub, 0:128]
                    v_hi = v_sb[ttk][:, ksub, 128:256]
                    nc.tensor.matmul(pos_t[0][:, lo:hi], v_lo,
                                     pt[:, 0, lo:hi], start=st, stop=sp,
                                     skip_group_check=True)
                    nc.tensor.matmul(pos_t[2][:, lo:hi], v_lo,
                                     pt[:, 1, lo:hi], start=st, stop=sp,
                                     skip_group_check=True)
                    nc.tensor.matmul(pos_t[1][:, lo:hi], v_hi,
                                     pt[:, 0, lo:hi], start=st, stop=sp,
                                     skip_group_check=True)
                    nc.tensor.matmul(pos_t[3][:, lo:hi], v_hi,
                                     pt[:, 1, lo:hi], start=st, stop=sp,
                                     skip_group_check=True)

                LOOK = 2
                for i in range(bst["emitted"], min(LOOK, n)):
                    scores(i)
                budget = 0.0
                for i in range(n):
                    if i + LOOK < n and i + LOOK >= bst["emitted"]:
                        scores(i + LOOK)
                    av(i)
                    budget += 28.0 / (n if last else max(n - 2, 1))
                    while budget >= 1.0:
                        next(cgen, None)
                        budget -= 1.0
                norm_src[(qb, 0)] = (pos_t[0], pos_t[1], dacc[:, 0, :])
                norm_src[(qb, 1)] = (pos_t[2], pos_t[3], dacc[:, 1, :])

            pre = prefetch(0)
            for tt in range(TT):
                bst = emit_A(tt, pre)
                if tt == 0:
                    # deferred low-priority loads (needed from C(0) on)
                    for fs in range(4):
                        nc.sync.dma_start(wo_sb[:, fs, :], woT_r[:, fs, :])
                if tt + 1 < TT:
                    pre = prefetch(tt + 1)
                cgen = emit_C_chunks(tt - 1) if tt > 0 else iter(())
                emit_B(bst, cgen, last=(tt == TT - 1))
                for _ in cgen:
                    pass
            emit_norm_stage1(TT - 1)
            emit_norm_stage2(TT - 1)
            for _ in emit_C_chunks(TT - 1, tail=True):
                pass

    nc.compile()
    return nc


def get_nc():
    if "nc" not in _NC_CACHE:
        _NC_CACHE["nc"] = build_nc()
    return _NC_CACHE["nc"]


def prep_in_maps(inputs):
    bf16 = ml_dtypes.bfloat16
    hs = np.asarray(inputs["hidden_states"], dtype=np.float32)
    pos = np.asarray(inputs["position_ids"]).reshape(-1).astype(np.float64)
    w_qkv = np.asarray(inputs["w_qkv"], dtype=np.float32)
    w_o = np.asarray(inputs["w_o"], dtype=np.float32)

    # hidTp[p, th, ko, q] = hs[256*th + q, 128*ko + p]
    hidTp = np.ascontiguousarray(
        hs.reshape(2 * TT, 256, KO, 128).astype(bf16).transpose(3, 0, 2, 1)
    )

    inv_freq = 1.0 / (THETA ** (np.arange(HD // 2, dtype=np.float64) * 2.0 / HD))
    ang = inv_freq[:, None] * pos[None, :]
    cosT = np.cos(ang).astype(np.float32)
    sinT = np.sin(ang).astype(np.float32)

    kk = np.arange(128)[:, None]
    qq = np.arange(512)[None, :]
    masksp = np.stack(
        [((qq - kk + o >= 0) & (qq - kk + o <= WINDOW)) for o in MASK_OFFS],
        axis=1,
    ).astype(bf16)  # [128, 8, 512]

    in_maps = []
    for c in range(N_CORES):
        wq = w_qkv[512 * c:512 * (c + 1)]
        wk = w_qkv[Q_SIZE + HD * c:Q_SIZE + HD * (c + 1)]
        wv = w_qkv[Q_SIZE + NKV * HD + HD * c:Q_SIZE + NKV * HD + HD * (c + 1)]
        # [p, pr, ko, f2] = W[256*pr + f2, 128*ko + p]
        wqk = np.concatenate([wq, wk], 0)  # [768, HID]
        wqkTp = np.ascontiguousarray(
            wqk.reshape(3, 256, KO, 128).astype(bf16).transpose(3, 0, 2, 1))
        wvTp = np.ascontiguousarray(
            wv.reshape(HD, KO, 128).astype(bf16).transpose(2, 1, 0))
        # [p, fs, h] = w_o[h, 512*c + 128*fs + p]
        woTp = np.ascontiguousarray(
            w_o[:, 512 * c:512 * (c + 1)].T
            .reshape(4, 128, HID).astype(bf16).transpose(1, 0, 2))
        in_maps.append(
            dict(hidTp=hidTp, wqkTp=wqkTp, wvTp=wvTp, woTp=woTp,
                 cosT=cosT, sinT=sinT, masksp=masksp)
        )
    return in_maps


def run(inputs, **kwargs):
    nc = get_nc()
    in_maps = prep_in_maps(inputs)
    return run_bass_kernel_spmd(nc, in_maps, list(range(N_CORES)), **kwargs)


def gather_results(res):
    """Sum the 8 full-shape bf16 partials (unshard of sum-sharded output)."""
    acc = np.zeros((S, HID), dtype=np.float64)
    for c in range(N_CORES):
        acc += np.asarray(res.results[c]["out"], dtype=np.float64)
    return acc.astype(np.float32).reshape(1, S, HID)


def kernel(**inputs):
    res = run(inputs)
    return gather_results(res)


# revision 28
# speedup vs baseline: 1.0104x; 1.0017x over previous
"""Gemma2 sliding-window attention (B=1, S=4096, HID=3584, 16 Q heads / 8 KV heads,
HD=256, window 2047, tanh softcap 50) on 8 Trainium2 NeuronCores.

Sharding: tensor-parallel over heads with NO on-device collectives. Core c owns
Q heads (2c, 2c+1) and KV head c, and computes a full-shape PARTIAL of the
output projection restricted to its own 512 attention features:
    partial_c = attn[:, 512c:512c+512] @ w_o[:, 512c:512c+512].T   [S, HID]
The host sums the 8 bf16 partials in float64 (unshard of the sum-sharded
output). This removes the AllGather + serial o-proj tail.

Per-core fused pipeline over 512-token tiles tt=0..7:
  A(tt): QKV projection (transposed for Q/K, straight for V) + NeoX RoPE.
         The normalization chain of block tt-1 (denominator matmuls -> fast
         approximate reciprocals -> partition broadcasts -> ao multiplies) is
         injected between A's projection groups, where the tensor queue is
         deep and the DVE dacc chain has long drained - it never stalls
         anything.
  B(tt): sliding-window attention for query block tt with BOTH heads merged
         per key-subtile: one combined [128, 2, 512] PSUM score tile (the
         head axis is the bank boundary), ONE batched tanh and ONE batched
         exp over both heads (amortizes the 352-cycle ACT instruction
         overhead - ACT was the B-phase bottleneck engine), shared K/V
         stationary operands, and per-element-has_written column-restricted
         boundary tiles (saves ~15% of score/PV/tanh/exp work). The
         denominator accumulates on DVE in bf16 (2x mode). o-proj chunks of
         block tt-1 interleave into the loop; their PSUM->SBUF copies ride
         the DVE (ACT stays reserved for the softmax chain).

Pipelining details:
  - Block tt's first two score batches are pre-emitted into A(tt)'s tail so
    their tanh/exp chain hides under the V-projection matmuls (no block-entry
    ACT bubble).
  - V-projection PSUMs ride the psO ring (free during A after the previous
    block's normalization reads), keeping the psS ring exclusively for the
    projection/score/o-proj rotation.
  - RoPE multiplies are ordered so each projection PSUM buffer is released
    after 2 DVE ops instead of 5.
  - o-proj chunk pacing finishes two iterations before each block ends so
    the next A phase never waits on a chunk-copy drain.
Startup: tile-0 DMAs are issued in just-in-time consumption order (hid/wqk
ko-chunked and interleaved on the sync ring; cos/sin/masks/hid-half1 on the
scalar HWDGE ring) so the first matmul starts ~4us in. The final o-proj
block alternates its copies across ACT and DVE (both idle by then) to
shorten the tail.

Measured on hardware: ~848 us (baseline of this session: 994 us), rel err
7.3e-3 vs the f64 reference; tensor engine ~94% occupied.
"""

import sys

if "/opt/trn_rl_repo" not in sys.path:
    sys.path.insert(0, "/opt/trn_rl_repo")

import numpy as np
import ml_dtypes

import concourse.bass as bass
import concourse.tile as tile
from concourse import bacc, mybir
from concourse.bass_utils import run_bass_kernel_spmd

# Problem constants (hardcoded per harness contract)
S = 4096
HID = 3584
NH, NKV, HD = 16, 8, 256
Q_SIZE = NH * HD          # 4096
SCALE = 256.0 ** -0.5     # 1/16
SOFTCAP = 50.0
WINDOW = 2048 - 1         # 2047
THETA = 10000.0

N_CORES = 8
KO = HID // 128           # 28 contraction subtiles for projections
TT = S // 512             # 8 token tiles of 512
HC = HID // 512           # 7 output-column chunks of 512
F32 = mybir.dt.float32
BF16 = mybir.dt.bfloat16

# Boundary-tile diagonal offsets (q0 - 128*kt). Interior iff 128 <= off <= 1536.
MASK_OFFS = [-384, -256, -128, 0, 1664, 1792, 1920, 2048]

_NC_CACHE = {}


def build_nc():
    nc = bacc.Bacc()

    # All inputs are host-side pre-arranged to [128 partitions, ...contiguous]
    # so every DMA is 128 large contiguous descriptors.
    hidT_r = nc.declare_dram_parameter("hidTp", [128, 2 * TT, KO, 256], BF16,
                                       isOutput=False)
    wqkT_r = nc.declare_dram_parameter("wqkTp", [128, 3, KO, 256], BF16,
                                       isOutput=False)
    wvT_r = nc.declare_dram_parameter("wvTp", [128, KO, HD], BF16,
                                      isOutput=False)
    woT_r = nc.declare_dram_parameter("woTp", [128, 4, HID], BF16,
                                      isOutput=False)
    cosT = nc.declare_dram_parameter("cosT", [128, S], F32, isOutput=False)
    sinT = nc.declare_dram_parameter("sinT", [128, S], F32, isOutput=False)
    masks_r = nc.declare_dram_parameter("masksp", [128, 8, 512], BF16,
                                        isOutput=False)
    out = nc.declare_dram_parameter("out", [S, HID], BF16, isOutput=True)

    with tile.TileContext(nc) as tc:
        with (
            tc.tile_pool(name="persist", bufs=1) as persist,
            tc.tile_pool(name="hidp", bufs=2) as hid_pool,
            tc.tile_pool(name="cs", bufs=2) as cs_pool,
            tc.tile_pool(name="qp", bufs=2) as q_pool,
            tc.tile_pool(name="rp", bufs=4) as rp_pool,
            tc.tile_pool(name="probs", bufs=4) as probs_pool,
            tc.tile_pool(name="aop", bufs=8) as ao_pool,
            tc.tile_pool(name="otp", bufs=5) as out_pool,
            tc.tile_pool(name="small", bufs=2) as small_pool,
            tc.tile_pool(name="dap", bufs=2) as dacc_pool,
            tc.tile_pool(name="psS", bufs=2, space="PSUM") as psS,
            tc.tile_pool(name="psO", bufs=4, space="PSUM") as psO,
        ):
            # Persistent SBUF: weights, per-tile K/V, masks, ones.
            wqk_sb = persist.tile([128, 3, KO, 256], BF16, tag="wqk")
            wv_sb = persist.tile([128, KO, HD], BF16, tag="wv")
            mask_sb = persist.tile([128, 8, 512], BF16, tag="mask")
            wo_sb = persist.tile([128, 4, HID], BF16, tag="wo")
            ones_bf = persist.tile([128, 1], BF16, tag="ones")
            nc.vector.memset(ones_bf, 1.0)
            # HAM warm-up: dependency-free dummy matmuls keep the PE busy
            # through the startup DMA waits so the real projections run at
            # the unthrottled 2.4 GHz clock from the start
            warm_sb = persist.tile([128, 512], BF16, tag="warm")
            nc.vector.memset(warm_sb, 0.0)
            warm_ps = psO.tile([128, 512], F32, tag="po", name="warm")
            for _ in range(16):
                nc.tensor.matmul(warm_ps, warm_sb[:, 0:128], warm_sb,
                                 start=True, stop=True,
                                 skip_group_check=True)

            k_sb = [persist.tile([128, 2, 512], BF16, tag=f"k{t}", name=f"k{t}")
                    for t in range(TT)]
            v_sb = [persist.tile([128, 4, HD], BF16, tag=f"v{t}", name=f"v{t}")
                    for t in range(TT)]

            ao_store = {}
            norm_src = {}
            norm_cell = {}

            def emit_norm_stage1(qb):
                """Denominator matmul + fast reciprocal + broadcast for both
                heads of block qb. Called mid-A(qb+1): the tensor queue is
                deep with projection matmuls, so the pden matmuls never
                head-of-line block, and the dacc chains drained long ago."""
                for h in (0, 1):
                    po_lo, po_hi, dacc_h = norm_src[(qb, h)]
                    pden = psS.tile([1, 512], F32, tag="s", name="pden")
                    nc.tensor.matmul(pden, ones_bf, dacc_h,
                                     start=True, stop=True,
                                     skip_group_check=True)
                    recip = small_pool.tile([1, 512], F32, tag="recip",
                                            name="recip")
                    nc.vector.reciprocal_approx_fast(recip, pden)
                    rb = small_pool.tile([128, 512], F32, tag="rb", name="rb")
                    nc.gpsimd.partition_broadcast(rb, recip)
                    norm_cell[h] = rb

            def emit_norm_stage2(qb):
                for h in (0, 1):
                    po_lo, po_hi, _ = norm_src.pop((qb, h))
                    rb = norm_cell.pop(h)
                    ao0 = ao_pool.tile([128, 512], BF16, tag="ao", name="ao0")
                    ao1 = ao_pool.tile([128, 512], BF16, tag="ao", name="ao1")
                    nc.vector.tensor_mul(ao0, po_lo, rb)
                    nc.vector.tensor_mul(ao1, po_hi, rb)
                    ao_store[(qb, h)] = (ao0, ao1)

            def prefetch(tt):
                """Issue hid/cos/sin DMAs for tile tt. Tile 0 gets a
                just-in-time order: hid/wqk ko-chunked and interleaved so the
                first matmul group is gated on ~1.4 MB, not 6 MB."""
                hids = []
                if tt == 0:
                    hid_h0 = hid_pool.tile([128, KO, 256], BF16, tag="hid",
                                           name="hid_h")
                    # fine-grained ko-interleave: each 7-ko chunk of hid and
                    # wqk pair 0 lands just before the projection consumes it
                    for c0 in range(0, KO, 7):
                        c1 = min(c0 + 7, KO)
                        nc.sync.dma_start(hid_h0[:, c0:c1, :],
                                          hidT_r[:, 0, c0:c1, :])
                        nc.sync.dma_start(wqk_sb[:, 0, c0:c1, :],
                                          wqkT_r[:, 0, c0:c1, :])
                    nc.sync.dma_start(wqk_sb[:, 1, 0:14, :],
                                      wqkT_r[:, 1, 0:14, :])
                    nc.sync.dma_start(wqk_sb[:, 1, 14:KO, :],
                                      wqkT_r[:, 1, 14:KO, :])
                    hids.append(hid_h0)
                    # small/late tile-0 transfers ride the second HWDGE
                    # ring (scalar): the sync-ring weight stream lands sooner
                    cos_t = cs_pool.tile([128, 512], F32, tag="cos",
                                         name="cos_t")
                    nc.scalar.dma_start(cos_t, cosT[:, bass.ts(tt, 512)])
                    sin_t = cs_pool.tile([128, 512], F32, tag="sin",
                                         name="sin_t")
                    nc.scalar.dma_start(sin_t, sinT[:, bass.ts(tt, 512)])
                    # second hid half + remaining weights are emitted JIT
                    # inside emit_A(0)
                    hid_h1 = hid_pool.tile([128, KO, 256], BF16, tag="hid",
                                           name="hid_h")
                    hids.append(hid_h1)
                    return hids, cos_t, sin_t, True
                for half in range(2):
                    hid_h = hid_pool.tile([128, KO, 256], BF16, tag="hid",
                                          name="hid_h")
                    nc.sync.dma_start(hid_h, hidT_r[:, tt * 2 + half, :, :])
                    hids.append(hid_h)
                cos_t = cs_pool.tile([128, 512], F32, tag="cos", name="cos_t")
                nc.sync.dma_start(cos_t, cosT[:, bass.ts(tt, 512)])
                sin_t = cs_pool.tile([128, 512], F32, tag="sin", name="sin_t")
                nc.sync.dma_start(sin_t, sinT[:, bass.ts(tt, 512)])
                return hids, cos_t, sin_t, False

            def make_B_state(qb, q_t):
                """Closures + state for block qb's merged-head attention.
                scores() is usable as soon as q RoPE is done, so the first
                two batches can be pre-emitted into A's tail (their tanh/exp
                hide under the V-projection matmuls, removing the block-entry
                ACT bubble)."""
                q0 = qb * 512
                kts = list(range(max(0, 4 * qb - 16), 4 * qb + 4))
                n = len(kts)
                dacc = dacc_pool.tile([128, 2, 512], BF16, tag="da",
                                      name="dacc")
                nc.vector.memset(dacc, 0.0)
                probs = {}
                st = dict(qb=qb, kts=kts, n=n, dacc=dacc, probs=probs,
                          emitted=0, pos_t=None)

                def scores(i):
                    kt = kts[i]
                    off = q0 - 128 * kt
                    lo, hi = max(0, -off), min(512, 2176 - off)
                    ttk, ksub = kt // 4, kt % 4
                    ksl = bass.ts(ksub, 128)
                    ps = psS.tile([128, 2, 512], F32, tag="s", name="ps_s")
                    nc.tensor.matmul(ps[:, 0, lo:hi], k_sb[ttk][:, 0, ksl],
                                     q_t[:, 0, lo:hi], start=True, stop=False)
                    nc.tensor.matmul(ps[:, 1, lo:hi], k_sb[ttk][:, 0, ksl],
                                     q_t[:, 2, lo:hi], start=True, stop=False)
                    nc.tensor.matmul(ps[:, 0, lo:hi], k_sb[ttk][:, 1, ksl],
                                     q_t[:, 1, lo:hi], start=False, stop=True)
                    nc.tensor.matmul(ps[:, 1, lo:hi], k_sb[ttk][:, 1, ksl],
                                     q_t[:, 3, lo:hi], start=False, stop=True)
                    pt = probs_pool.tile([128, 2, 512], BF16, tag="pt",
                                         name="pt")
                    nc.scalar.activation(
                        ps[:, :, lo:hi], ps[:, :, lo:hi],
                        mybir.ActivationFunctionType.Tanh,
                        scale=SCALE / SOFTCAP,
                    )
                    nc.scalar.activation(
                        pt[:, :, lo:hi], ps[:, :, lo:hi],
                        mybir.ActivationFunctionType.Exp,
                        scale=SOFTCAP,
                    )
                    if not (128 <= off <= 1536):
                        mi = MASK_OFFS.index(off)
                        for hh in (0, 1):
                            nc.vector.tensor_mul(pt[:, hh, lo:hi],
                                                 pt[:, hh, lo:hi],
                                                 mask_sb[:, mi, lo:hi])
                    if lo == 0 and hi == 512:
                        nc.vector.tensor_add(dacc, dacc, pt)
                    else:
                        for hh in (0, 1):
                            nc.vector.tensor_add(dacc[:, hh, lo:hi],
                                                 dacc[:, hh, lo:hi],
                                                 pt[:, hh, lo:hi])
                    probs[i] = pt

                st["scores"] = scores
                return st

            def emit_A(tt, pre):
                """QKV projection + RoPE for token tile tt. Returns the B
                state for block tt. Block tt-1's normalization stages are
                injected between projection groups of the first half; block
                tt's first two score batches are pre-emitted before the
                second half's V-projection groups."""
                hids, cos_t, sin_t, jit0 = pre
                q_t = q_pool.tile([128, 4, 512], BF16, tag="q", name="q_t")
                bst = None
                for half in range(2):
                    csl = bass.ts(half, 256)
                    hid_h = hids[half]
                    for pair in range(3):
                        if jit0 and half == 0 and pair == 2:
                            nc.sync.dma_start(wqk_sb[:, 2, 0:14, :],
                                              wqkT_r[:, 2, 0:14, :])
                            nc.sync.dma_start(wqk_sb[:, 2, 14:KO, :],
                                              wqkT_r[:, 2, 14:KO, :])
                        if jit0 and half == 0 and pair == 1:
                            # fill the pair0->pair1 weight-DMA wait
                            for _ in range(20):
                                nc.tensor.matmul(warm_ps, warm_sb[:, 0:128],
                                                 warm_sb, start=True,
                                                 stop=True,
                                                 skip_group_check=True)
                        if half == 0 and pair == 2 and tt > 0:
                            emit_norm_stage1(tt - 1)
                        ps_a = psS.tile([128, 256], F32, tag="s", name="ps_a")
                        for ko in range(KO):
                            nc.tensor.matmul(
                                ps_a,
                                wqk_sb[:, pair, ko, 0:128],
                                hid_h[:, ko, :],
                                start=(ko == 0), stop=(ko == KO - 1),
                            )
                        ps_b = psS.tile([128, 256], F32, tag="s", name="ps_b")
                        for ko in range(KO):
                            nc.tensor.matmul(
                                ps_b,
                                wqk_sb[:, pair, ko, 128:256],
                                hid_h[:, ko, :],
                                start=(ko == 0), stop=(ko == KO - 1),
                            )
                        if pair < 2:
                            d1 = q_t[:, 2 * pair, csl]
                            d2 = q_t[:, 2 * pair + 1, csl]
                        else:
                            d1 = k_sb[tt][:, 0, csl]
                            d2 = k_sb[tt][:, 1, csl]
                        # ps_a is read by the first two DVE ops and ps_b by
                        # the next two, so each PSUM ring buffer frees after
                        # 2 ops instead of 5 - the next projection group's
                        # matmuls unblock ~1.5us earlier
                        t1 = rp_pool.tile([128, 256], F32, tag="rp", name="t1")
                        t4 = rp_pool.tile([128, 256], F32, tag="rp", name="t4")
                        nc.vector.tensor_mul(t1, ps_a, cos_t[:, csl])
                        nc.vector.tensor_mul(t4, ps_a, sin_t[:, csl])
                        t2 = rp_pool.tile([128, 256], F32, tag="rp", name="t2")
                        t3 = rp_pool.tile([128, 256], F32, tag="rp", name="t3")
                        nc.vector.tensor_mul(t2, ps_b, sin_t[:, csl])
                        nc.vector.tensor_mul(t3, ps_b, cos_t[:, csl])
                        nc.vector.tensor_sub(d1, t1, t2)
                        nc.vector.tensor_add(d2, t3, t4)
                    if half == 0 and tt > 0:
                        emit_norm_stage2(tt - 1)
                    if jit0 and half == 0:
                        nc.sync.dma_start(wv_sb, wvT_r[:, :, :])
                        nc.scalar.dma_start(hids[1], hidT_r[:, 1, :, :])
                    if half == 1:
                        if jit0:
                            # the prologue below multiplies by the boundary
                            # masks - their load must be emitted first
                            nc.scalar.dma_start(mask_sb, masks_r[:, :, :])
                        # pre-emit block tt's first two score batches: q RoPE
                        # is complete; their ACT chain overlaps the V matmuls
                        bst = make_B_state(tt, q_t)
                        bst["scores"](0)
                        bst["scores"](1)
                        bst["emitted"] = 2
                    for j in range(2):
                        ps_v = psO.tile([128, HD], F32, tag="po", name="ps_v")
                        for ko in range(KO):
                            nc.tensor.matmul(
                                ps_v,
                                hid_h[:, ko, bass.ts(j, 128)],
                                wv_sb[:, ko, :],
                                start=(ko == 0), stop=(ko == KO - 1),
                            )
                        nc.scalar.copy(v_sb[tt][:, half * 2 + j, :], ps_v)
                # third score batch after the V groups: it sits behind the V
                # matmuls in the tensor queue (never blocks them) and its
                # PSUM-ring wait resolves during the V window, so block entry
                # starts with three batches of ACT-chain slack
                if bst["n"] > 2:
                    bst["scores"](2)
                    bst["emitted"] = 3
                return bst

            def emit_C_chunks(qb, tail=False):
                """o-proj partial for query block qb: 28 chunk generators.
                In-loop copies ride DVE (ACT owns the softmax chain); the
                final block alternates ACT/DVE since both are idle then."""
                ao_h0 = ao_store.pop((qb, 0))
                ao_h1 = ao_store.pop((qb, 1))
                aos = [ao_h0[0], ao_h0[1], ao_h1[0], ao_h1[1]]
                idx = 0
                for tsub in range(4):
                    for hc in range(HC):
                        ps = psS.tile([128, 512], F32, tag="s", name="psC")
                        for fs in range(4):
                            nc.tensor.matmul(
                                ps,
                                aos[fs][:, bass.ts(tsub, 128)],
                                wo_sb[:, fs, bass.ts(hc, 512)],
                                start=(fs == 0), stop=(fs == 3),
                                skip_group_check=True,
                            )
                        ot = out_pool.tile([128, 512], BF16, tag="ot",
                                           name="ot")
                        if tail and idx % 2 == 0:
                            nc.scalar.copy(ot, ps)
                        else:
                            nc.vector.tensor_scalar_add(ot, ps, 0.0)
                        r0 = qb * 512 + tsub * 128
                        nc.sync.dma_start(
                            out[r0:r0 + 128, bass.ts(hc, 512)], ot
                        )
                        idx += 1
                        yield

            def emit_B(bst, cgen, last=False):
                """Merged-head attention for one query block. Both heads
                share each key-subtile's stationary K/V operands; tanh/exp
                run once over the combined [128, 2, w] region. Boundary key
                subtiles are restricted to their live q-column range
                [lo, hi); PV accumulation relies on per-element PSUM
                has_written bits. o-proj chunk pacing finishes two
                iterations before the block ends so the next A phase never
                waits on a chunk-copy drain."""
                qb, kts, n = bst["qb"], bst["kts"], bst["n"]
                q0 = qb * 512
                dacc, probs, scores = bst["dacc"], bst["probs"], bst["scores"]
                pos_t = [psO.tile([128, 512], F32, tag="po", name=f"po{j}")
                         for j in range(4)]

                def av(i):
                    kt = kts[i]
                    off = q0 - 128 * kt
                    lo, hi = max(0, -off), min(512, 2176 - off)
                    ttk, ksub = kt // 4, kt % 4
                    pt = probs.pop(i)
                    st, sp = (i == 0), (i == n - 1)
                    v_lo = v_sb[ttk][:, ksub, 0:128]
                    v_hi = v_sb[ttk][:, ksub, 128:256]
                    nc.tensor.matmul(pos_t[0][:, lo:hi], v_lo,
                                     pt[:, 0, lo:hi], start=st, stop=sp,
                                     skip_group_check=True)
                    nc.tensor.matmul(pos_t[2][:, lo:hi], v_lo,
                                     pt[:, 1, lo:hi], start=st, stop=sp,
                                     skip_group_check=True)
                    nc.tensor.matmul(pos_t[1][:, lo:hi], v_hi,
                                     pt[:, 0, lo:hi], start=st, stop=sp,
                                     skip_group_check=True)
                    nc.tensor.matmul(pos_t[3][:, lo:hi], v_hi,
                                     pt[:, 1, lo:hi], start=st, stop=sp,
                                     skip_group_check=True)

                LOOK = 2
                for i in range(bst["emitted"], min(LOOK, n)):
                    scores(i)
                budget = 0.0
                for i in range(n):
                    if i + LOOK < n and i + LOOK >= bst["emitted"]:
                        scores(i + LOOK)
                    av(i)
                    budget += 28.0 / (n if last else max(n - 2, 1))
                    while budget >= 1.0:
                        next(cgen, None)
                        budget -= 1.0
                norm_src[(qb, 0)] = (pos_t[0], pos_t[1], dacc[:, 0, :])
                norm_src[(qb, 1)] = (pos_t[2], pos_t[3], dacc[:, 1, :])

            pre = prefetch(0)
            for tt in range(TT):
                bst = emit_A(tt, pre)
                if tt == 0:
                    # deferred low-priority loads (needed from C(0) on)
                    for fs in range(4):
                        nc.sync.dma_start(wo_sb[:, fs, :], woT_r[:, fs, :])
                if tt + 1 < TT:
                    pre = prefetch(tt + 1)
                cgen = emit_C_chunks(tt - 1) if tt > 0 else iter(())
                emit_B(bst, cgen, last=(tt == TT - 1))
                for _ in cgen:
                    pass
            emit_norm_stage1(TT - 1)
            emit_norm_stage2(TT - 1)
            for _ in emit_C_chunks(TT - 1, tail=True):
                pass

    nc.compile()
    return nc


def get_nc():
    if "nc" not in _NC_CACHE:
        _NC_CACHE["nc"] = build_nc()
    return _NC_CACHE["nc"]


def prep_in_maps(inputs):
    bf16 = ml_dtypes.bfloat16
    hs = np.asarray(inputs["hidden_states"], dtype=np.float32)
    pos = np.asarray(inputs["position_ids"]).reshape(-1).astype(np.float64)
    w_qkv = np.asarray(inputs["w_qkv"], dtype=np.float32)
    w_o = np.asarray(inputs["w_o"], dtype=np.float32)

    # hidTp[p, th, ko, q] = hs[256*th + q, 128*ko + p]
    hidTp = np.ascontiguousarray(
        hs.reshape(2 * TT, 256, KO, 128).astype(bf16).transpose(3, 0, 2, 1)
    )

    inv_freq = 1.0 / (THETA ** (np.arange(HD // 2, dtype=np.float64) * 2.0 / HD))
    ang = inv_freq[:, None] * pos[None, :]
    cosT = np.cos(ang).astype(np.float32)
    sinT = np.sin(ang).astype(np.float32)

    kk = np.arange(128)[:, None]
    qq = np.arange(512)[None, :]
    masksp = np.stack(
        [((qq - kk + o >= 0) & (qq - kk + o <= WINDOW)) for o in MASK_OFFS],
        axis=1,
    ).astype(bf16)  # [128, 8, 512]

    in_maps = []
    for c in range(N_CORES):
        wq = w_qkv[512 * c:512 * (c + 1)]
        wk = w_qkv[Q_SIZE + HD * c:Q_SIZE + HD * (c + 1)]
        wv = w_qkv[Q_SIZE + NKV * HD + HD * c:Q_SIZE + NKV * HD + HD * (c + 1)]
        # [p, pr, ko, f2] = W[256*pr + f2, 128*ko + p]
        wqk = np.concatenate([wq, wk], 0)  # [768, HID]
        wqkTp = np.ascontiguousarray(
            wqk.reshape(3, 256, KO, 128).astype(bf16).transpose(3, 0, 2, 1))
        wvTp = np.ascontiguousarray(
            wv.reshape(HD, KO, 128).astype(bf16).transpose(2, 1, 0))
        # [p, fs, h] = w_o[h, 512*c + 128*fs + p]
        woTp = np.ascontiguousarray(
            w_o[:, 512 * c:512 * (c + 1)].T
            .reshape(4, 128, HID).astype(bf16).transpose(1, 0, 2))
        in_maps.append(
            dict(hidTp=hidTp, wqkTp=wqkTp, wvTp=wvTp, woTp=woTp,
                 cosT=cosT, sinT=sinT, masksp=masksp)
        )
    return in_maps


def run(inputs, **kwargs):
    nc = get_nc()
    in_maps = prep_in_maps(inputs)
    return run_bass_kernel_spmd(nc, in_maps, list(range(N_CORES)), **kwargs)


def gather_results(res):
    """Sum the 8 full-shape bf16 partials (unshard of sum-sharded output)."""
    acc = np.zeros((S, HID), dtype=np.float64)
    for c in range(N_CORES):
        acc += np.asarray(res.results[c]["out"], dtype=np.float64)
    return acc.astype(np.float32).reshape(1, S, HID)


def kernel(**inputs):
    res = run(inputs)
    return gather_results(res)
